# revision 1
# baseline (speedup 1.0000x reference)
"""Trainium2 Bass kernel for nn_ConnectionTransformer (8 NeuronCores, SPMD).

Strategy
--------
- Phase A (embed + compress attention): batch-parallel, core c handles batch c.
- Phase B (6 bilinear message-passing steps): target-slot sharding — core c owns
  16 target slots j in [16c, 16c+16). Each core computes the full influence for
  its slots (sum over all source slots i), applies relu/residual/LayerNorm
  locally, and an AllGather rebuilds the replicated transposed state h^T each
  step. The per-pair weights W_source/W_target (1 GB total) are sharded along j
  and streamed from HBM once per step per core (64+64 MB) — the memory roofline.
- Phase C (expand attention + vocab projection): batch-parallel again.

All weights are pre-transposed/tiled on the host into matmul-ready layouts so
the device never transposes weight tensors.
"""
import os
import sys

sys.path.insert(0, "/opt/trn_rl_repo")

import numpy as np
from concourse import bass, bacc, tile, bass_utils, mybir
from concourse import masks

B, L, D, S, R, STEPS, V = 8, 512, 256, 128, 32, 6, 32000
NC = 8
JL = S // NC          # 16 local target slots per core
K = JL // 4           # 4 quads of target slots
VT = 500              # vocab tile width
NVT = V // VT         # 64 vocab tiles
SCALE = 1.0 / np.sqrt(D)
LN_EPS = 1e-5

F32 = mybir.dt.float32
F32R = mybir.dt.float32r
I16 = mybir.dt.int16

# "f32" (exact) or "f32r" (full-rate fp32 on the PE for the big einsums)
MM_MODE = os.environ.get("MM_MODE", "f32")
N_STEPS = int(os.environ.get("N_STEPS", str(STEPS)))


def _mm(ap):
    """Bitcast an AP to float32r when running the big einsums in f32r mode."""
    if MM_MODE == "f32r":
        return ap.bitcast(F32R)
    return ap


# ---------------------------------------------------------------------------
# Device program
# ---------------------------------------------------------------------------

def build():
    nc = bacc.Bacc("TRN2", target_bir_lowering=False, debug=False, num_devices=NC)

    io = {}

    def inp(name, shape, dtype=F32):
        io[name] = nc.dram_tensor(name, shape, dtype, kind="ExternalInput").ap()

    inp("xT_in", [D, L])
    inp("maskw", [128, 4])
    for w in ("wqT", "wkslT", "wvT", "wqoT", "wkfT", "wvfT"):
        inp(w, [D, D])
    inp("hT_in", [D, S])
    inp("hn_in", [S, D])
    inp("lngb", [B, STEPS, 2 * D])
    inp("wsrc", [S, K, 2, 128, 4 * R])
    inp("wtgt", [K, 32, 4 * R, 4, D])
    inp("woutT", [NVT, 2, 128, VT])
    io["lg_out"] = nc.dram_tensor(
        "lg_out", [L, V], F32, kind="ExternalOutput"
    ).ap()
    io["dbg"] = nc.dram_tensor(
        "dbg", [128, 2048], F32, kind="ExternalOutput"
    ).ap()

    with tile.TileContext(nc) as tc:
        _body(nc, tc, io)
    nc.compile()
    return nc


def _body(nc, tc, io):
    with tc.tile_pool(name="const", bufs=1) as const, \
         tc.tile_pool(name="state", bufs=1) as state:

        ident = const.tile([128, 128], F32)
        masks.make_identity(nc, ident[:])
        ones = const.tile([128, 1], F32)
        nc.vector.memset(ones[:], 1.0)
        eps_sb = const.tile([128, 1], F32)
        nc.vector.memset(eps_sb[:], LN_EPS)

        pid = nc.sync.partition_id()

        # persistent state
        hT = [state.tile([128, S, B], F32, name=f"hT{dt}") for dt in range(2)]
        h_upd = state.tile([B, JL, D], F32)      # this core's 16 slots, all batches
        qoT = [state.tile([128, L], F32, name=f"qoT{pt}") for pt in range(2)]
        lngb_sb = state.tile([B, STEPS, 2 * D], F32)
        nc.sync.dma_start(lngb_sb[:], io["lngb"])

        _phase_a(nc, tc, io, ident, ones, pid, hT, h_upd, qoT)
        for t in range(N_STEPS):
            _step(nc, tc, t, io["wsrc"], io["wtgt"], hT, h_upd, lngb_sb, ident,
                  eps_sb)
        _phase_c(nc, tc, io, ident, pid, hT, qoT)


def _phase_a(nc, tc, io, ident, ones, pid, hT, h_upd, qoT):
    with tc.tile_pool(name="pa_sb", bufs=1) as pa, \
         tc.tile_pool(name="pa_ps", bufs=3, space="PSUM") as pps, \
         tc.tile_pool(name="pa_tp", bufs=2, space="PSUM") as tps, \
         tc.tile_pool(name="pa_acc", bufs=1, space="PSUM") as aps, \
         tc.tile_pool(name="dram_a", bufs=1, space="DRAM") as dra:

        mask_sb = pa.tile([128, 4], F32)
        nc.sync.dma_start(mask_sb[:], io["maskw"])

        # X^T tiles [d128, t512] (host-gathered embeddings, transposed)
        xT = [pa.tile([128, L], F32, name=f"xT{ct}") for ct in range(2)]
        for ct in range(2):
            nc.sync.dma_start(xT[ct][:], io["xT_in"][128 * ct : 128 * (ct + 1), :])

        # weight tiles [d128, 256] (contraction on partitions)
        def load_w(name):
            ts = [pa.tile([128, D], F32, name=f"{name}_{ct}") for ct in range(2)]
            for ct in range(2):
                nc.sync.dma_start(ts[ct][:], io[name][128 * ct : 128 * (ct + 1), :])
            return ts

        wq_sb = load_w("wqT")
        wv_sb = load_w("wvT")
        wksl_sb = load_w("wkslT")
        wqo_sb = load_w("wqoT")
        hTt = [pa.tile([128, S], F32, name=f"hTt{ct}") for ct in range(2)]
        for ct in range(2):
            nc.sync.dma_start(hTt[ct][:], io["hT_in"][128 * ct : 128 * (ct + 1), :])
        hn_sb = pa.tile([S, D], F32)
        nc.sync.dma_start(hn_sb[:], io["hn_in"])

        # Q_in^T and Q_out^T : [d'128 x 2, t512]
        qT = [pa.tile([128, L], F32, name=f"qT{pt}") for pt in range(2)]
        for pt in range(2):
            for dst, wsb in ((qT, wq_sb), (qoT, wqo_sb)):
                ps = pps.tile([128, L], F32, tag="ps")
                for ct in range(2):
                    nc.tensor.matmul(
                        ps[:], wsb[ct][:, 128 * pt : 128 * (pt + 1)], xT[ct][:],
                        start=(ct == 0), stop=(ct == 1),
                    )
                nc.vector.tensor_copy(dst[pt][:], ps[:])

        # V_in natural [t128 x 4, d256]
        vn = pa.tile([128, 4, D], F32)
        for tt in range(4):
            ps = pps.tile([128, L], F32, tag="ps")
            for ct in range(2):
                nc.tensor.matmul(
                    ps[:, 0:D], xT[ct][:, 128 * tt : 128 * (tt + 1)], wv_sb[ct][:],
                    start=(ct == 0), stop=(ct == 1),
                )
            nc.vector.tensor_copy(vn[:, tt, :], ps[:, 0:D])

        # K_slots^T [d'128 x 2, s128]
        kslT = [pa.tile([128, S], F32, name=f"kslT{pt}") for pt in range(2)]
        for pt in range(2):
            ps = pps.tile([128, L], F32, tag="ps")
            for ct in range(2):
                nc.tensor.matmul(
                    ps[:, 0:S], wksl_sb[ct][:, 128 * pt : 128 * (pt + 1)], hTt[ct][:],
                    start=(ct == 0), stop=(ct == 1),
                )
            nc.vector.tensor_copy(kslT[pt][:], ps[:, 0:S])

        # attention scores + masked softmax
        a_sb = pa.tile([128, 4, S], F32)
        for tt in range(4):
            sc = pps.tile([128, L], F32, tag="ps")
            for pt in range(2):
                nc.tensor.matmul(
                    sc[:, 0:S], qT[pt][:, 128 * tt : 128 * (tt + 1)], kslT[pt][:],
                    start=(pt == 0), stop=(pt == 1),
                )
            rowmax = pa.tile([128, 1], F32, tag="rmax")
            nc.vector.tensor_reduce(
                rowmax[:], sc[:, 0:S], axis=mybir.AxisListType.X,
                op=mybir.AluOpType.max,
            )
            nb = pa.tile([128, 1], F32, tag="nb")
            nc.vector.tensor_scalar_mul(nb[:], rowmax[:], -SCALE)
            sumexp = pa.tile([128, 1], F32, tag="sexp")
            nc.scalar.activation(
                a_sb[:, tt, :], sc[:, 0:S], mybir.ActivationFunctionType.Exp,
                bias=nb[:], scale=SCALE, accum_out=sumexp[:],
            )
            rs = pa.tile([128, 1], F32, tag="rs")
            nc.vector.reciprocal(rs[:], sumexp[:])
            rm = pa.tile([128, 1], F32, tag="rmk")
            nc.vector.tensor_tensor(
                rm[:], rs[:], mask_sb[:, tt : tt + 1], op=mybir.AluOpType.mult
            )
            nc.vector.tensor_scalar_mul(a_sb[:, tt, :], a_sb[:, tt, :], rm[:])

        # column sums and IR = A^T @ V
        cs = aps.tile([128, 1], F32, tag="cs")
        for tt in range(4):
            nc.tensor.matmul(
                cs[:], a_sb[:, tt, :], ones[:, 0:1], start=(tt == 0), stop=(tt == 3)
            )
        ir = aps.tile([128, D], F32, tag="ir")
        for tt in range(4):
            nc.tensor.matmul(
                ir[:], a_sb[:, tt, :], vn[:, tt, :], start=(tt == 0), stop=(tt == 3)
            )
        cssb = pa.tile([128, 1], F32)
        nc.vector.tensor_scalar_add(cssb[:], cs[:], 1e-8)
        rcs = pa.tile([128, 1], F32)
        nc.vector.reciprocal(rcs[:], cssb[:])
        h0 = pa.tile([S, D], F32)
        nc.vector.scalar_tensor_tensor(
            h0[:], ir[:], rcs[:], hn_sb[:],
            op0=mybir.AluOpType.mult, op1=mybir.AluOpType.add,
        )

        nc.sync.dma_start(io["dbg"][:, 1024:1280], h0[:])
        # h0 -> transposed bounce + natural bounce, init AllGather
        agin0 = dra.tile([4 * 16384], F32)
        for dt in range(2):
            p3 = tps.tile([128, 128], F32, tag="tp")
            nc.tensor.transpose(p3[:], h0[:, 128 * dt : 128 * (dt + 1)], ident[:])
            h0T = pa.tile([128, 128], F32, tag="h0T")
            nc.vector.tensor_copy(h0T[:], p3[:])
            nc.sync.dma_start(
                agin0[dt * 16384 : (dt + 1) * 16384].rearrange(
                    "(p f) -> p f", p=128
                ),
                h0T[:],
            )
        nc.sync.dma_start(
            agin0[32768:65536].rearrange("(p f) -> p f", p=128), h0[:]
        )
        agout0 = dra.tile([NC, 4 * 16384], F32, addr_space="Shared")
        nc.gpsimd.collective_compute(
            "AllGather", mybir.AluOpType.bypass,
            ins=[agin0[:].opt()], outs=[agout0[:].opt()],
            replica_groups=[list(range(NC))],
        )
        # readback: hT[dt][dp, s, b] ; h_upd[b, jl, d] (own slot range via pid)
        ag0r = agout0[:].rearrange(
            "b (seg dp s) -> seg dp s b", seg=4, dp=128, s=128
        )
        for dt in range(2):
            nc.sync.dma_start(hT[dt][:], ag0r[dt])
        nc.sync.dma_start(
            h_upd[:].rearrange("b jl d -> b (jl d)"),
            agout0[:][:, bass.ds(pid * (JL * D) + 32768, JL * D)],
        )


def _step(nc, tc, t, wsrc, wtgt, hT, h_upd, lngb_sb, ident, eps_sb):
    """One message-passing step."""
    with tc.tile_pool(name=f"s{t}_ws", bufs=4) as wsp, \
         tc.tile_pool(name=f"s{t}_wt", bufs=6) as wtp, \
         tc.tile_pool(name=f"s{t}_sb", bufs=1) as sb, \
         tc.tile_pool(name=f"s{t}_p1", bufs=2, space="PSUM") as p1p, \
         tc.tile_pool(name=f"s{t}_p2", bufs=1, space="PSUM") as p2p, \
         tc.tile_pool(name=f"s{t}_p3", bufs=2, space="PSUM") as p3p, \
         tc.tile_pool(name=f"s{t}_dram", bufs=1, space="DRAM") as drp:

        inter = sb.tile([128, K, S, B], F32)

        # ---- einsum1: inter[(jloc,r), k, i, b] = h[b,i,:] @ W_source[i, j] ----
        for i in range(S):
            ws = wsp.tile([128, K, 2, 4 * R], F32, tag="ws")
            nc.sync.dma_start(
                ws[:], wsrc[i].rearrange("k dt dp jr -> dp k dt jr")
            )
            p1 = p1p.tile([128, K, B], F32, tag="p1")
            for k in range(K):
                for dt in range(2):
                    nc.tensor.matmul(
                        p1[:, k, :],
                        _mm(ws[:, k, dt, :]),
                        _mm(hT[dt][:, i, :]),
                        start=(dt == 0), stop=(dt == 1),
                    )
            nc.vector.tensor_copy(inter[:, :, i, :], p1[:])

        # ---- einsum2 + relu/residual/LN per quad k ----
        hTloc = [sb.tile([128, JL * B], F32, name=f"hTl{dt}") for dt in range(2)]
        for k in range(K):
            p2 = [p2p.tile([B, D], F32, tag=f"p2_{jl}", name=f"p2_{jl}") for jl in range(4)]
            for g in range(32):
                wt = wtp.tile([128, 4, D], F32, tag="wt")
                nc.sync.dma_start(wt[:], wtgt[k, g])
                for il in range(4):
                    i = 4 * g + il
                    for jloc in range(4):
                        nc.tensor.matmul(
                            p2[jloc][:],
                            _mm(inter[32 * jloc : 32 * (jloc + 1), k, i, :]),
                            _mm(wt[32 * jloc : 32 * (jloc + 1), il, :]),
                            start=(i == 0), stop=(i == S - 1),
                            tile_position=(32 * jloc, 0),
                        )
            # relu, +h_old, LayerNorm over d for the 4 slots of this quad
            hrelu = sb.tile([B, 4, D], F32, tag="hrelu")
            for jloc in range(4):
                nc.scalar.activation(
                    hrelu[:, jloc, :], p2[jloc][:],
                    mybir.ActivationFunctionType.Relu,
                )
            hsum = sb.tile([B, 4, D], F32, tag="hsum")
            nc.vector.tensor_tensor(
                hsum[:], hrelu[:], h_upd[:, 4 * k : 4 * (k + 1), :],
                op=mybir.AluOpType.add,
            )
            mean = sb.tile([B, 4], F32, tag="mean")
            nc.vector.tensor_reduce(
                mean[:], hsum[:], axis=mybir.AxisListType.X, op=mybir.AluOpType.add
            )
            nc.vector.tensor_scalar_mul(mean[:], mean[:], 1.0 / D)
            cen = sb.tile([B, 4, D], F32, tag="cen")
            nc.vector.tensor_tensor(
                cen[:], hsum[:], mean[:].to_broadcast((B, 4, D)),
                op=mybir.AluOpType.subtract,
            )
            sq = sb.tile([B, 4, D], F32, tag="sq")
            nc.vector.tensor_tensor(
                sq[:], cen[:], cen[:], op=mybir.AluOpType.mult
            )
            var = sb.tile([B, 4], F32, tag="var")
            nc.vector.tensor_reduce(
                var[:], sq[:], axis=mybir.AxisListType.X, op=mybir.AluOpType.add
            )
            std = sb.tile([B, 4], F32, tag="std")
            nc.scalar.activation(
                std[:], var[:], mybir.ActivationFunctionType.Sqrt,
                bias=eps_sb[0:B, :], scale=1.0 / D,
            )
            rstd = sb.tile([B, 4], F32, tag="rstd")
            nc.vector.reciprocal(rstd[:], std[:])
            hnorm = sb.tile([B, 4, D], F32, tag="hnorm")
            nc.vector.tensor_tensor(
                hnorm[:], cen[:], rstd[:].to_broadcast((B, 4, D)),
                op=mybir.AluOpType.mult,
            )
            g_b = lngb_sb[:, t : t + 1, 0:D].to_broadcast((B, 4, D))
            b_b = lngb_sb[:, t : t + 1, D : 2 * D].to_broadcast((B, 4, D))
            nc.vector.tensor_tensor(
                hnorm[:], hnorm[:], g_b, op=mybir.AluOpType.mult
            )
            nc.vector.tensor_tensor(
                h_upd[:, 4 * k : 4 * (k + 1), :], hnorm[:], b_b,
                op=mybir.AluOpType.add,
            )
            # transpose the 4 updated slots into hTloc
            for jloc in range(4):
                jl = 4 * k + jloc
                for dt in range(2):
                    p3 = p3p.tile([128, B], F32, tag="p3")
                    nc.tensor.transpose(
                        p3[:],
                        h_upd[:, jl, 128 * dt : 128 * (dt + 1)],
                        ident[0:B, 0:B],
                    )
                    nc.vector.tensor_copy(
                        hTloc[dt][:, jl * B : (jl + 1) * B], p3[:]
                    )

        # ---- AllGather the transposed updated slots; rebuild hT ----
        agin = drp.tile([2 * 128 * JL * B], F32)
        for dt in range(2):
            nc.sync.dma_start(
                agin[dt * 16384 : (dt + 1) * 16384].rearrange(
                    "(p f) -> p f", p=128
                ),
                hTloc[dt][:],
            )
        agout = drp.tile([NC, 2 * 128 * JL * B], F32, addr_space="Shared")
        nc.gpsimd.collective_compute(
            "AllGather", mybir.AluOpType.bypass,
            ins=[agin[:].opt()], outs=[agout[:].opt()],
            replica_groups=[list(range(NC))],
        )
        agr = agout[:].rearrange(
            "rk (dt dp jl b) -> dt dp rk jl b", dt=2, dp=128, jl=JL, b=B
        )
        for dt in range(2):
            nc.sync.dma_start(
                hT[dt][:].rearrange("dp (rk jl) b -> dp rk jl b", rk=NC), agr[dt]
            )


def _phase_c(nc, tc, io, ident, pid, hT, qoT):
    with tc.tile_pool(name="pc_sb", bufs=1) as pc, \
         tc.tile_pool(name="pc_ps", bufs=3, space="PSUM") as cps, \
         tc.tile_pool(name="pc_lg", bufs=4, space="PSUM") as lgps, \
         tc.tile_pool(name="pc_wo", bufs=4) as wop:

        wkf_sb = [pc.tile([128, D], F32, name=f"wkf{ct}") for ct in range(2)]
        wvf_sb = [pc.tile([128, D], F32, name=f"wvf{ct}") for ct in range(2)]
        for ct in range(2):
            nc.sync.dma_start(
                wkf_sb[ct][:], io["wkfT"][128 * ct : 128 * (ct + 1), :]
            )
            nc.sync.dma_start(
                wvf_sb[ct][:], io["wvfT"][128 * ct : 128 * (ct + 1), :]
            )

        # own-batch h^T slice (dynamic b=pid) -> static tiles
        pid_v = nc.vector.partition_id()
        hb = [pc.tile([128, S], F32, name=f"hb{dt}") for dt in range(2)]
        for dt in range(2):
            nc.vector.tensor_copy(
                hb[dt][:].rearrange("p (s o) -> p s o", o=1),
                hT[dt][:, :, bass.ds(pid_v, 1)],
            )

        nc.sync.dma_start(io["dbg"][:, 1280:1408], hb[0][:])
        nc.sync.dma_start(io["dbg"][:, 1408:1536], hb[1][:])
        # K_f^T [d'128 x2, s128] ; V_f natural [s, d']
        kfT = [pc.tile([128, S], F32, name=f"kfT{pt}") for pt in range(2)]
        for pt in range(2):
            ps = cps.tile([128, L], F32, tag="c")
            for ct in range(2):
                nc.tensor.matmul(
                    ps[:, 0:S], wkf_sb[ct][:, 128 * pt : 128 * (pt + 1)], hb[ct][:],
                    start=(ct == 0), stop=(ct == 1),
                )
            nc.vector.tensor_copy(kfT[pt][:], ps[:, 0:S])
        vf = pc.tile([S, D], F32)
        psv = cps.tile([128, L], F32, tag="c")
        for ct in range(2):
            nc.tensor.matmul(
                psv[0:S, 0:D], hb[ct][:], wvf_sb[ct][:],
                start=(ct == 0), stop=(ct == 1),
            )
        nc.vector.tensor_copy(vf[:], psv[0:S, 0:D])

        # expand attention -> A2^T [s, t512]
        a2T = pc.tile([S, L], F32)
        for tt in range(4):
            sc = cps.tile([128, L], F32, tag="c")
            for pt in range(2):
                nc.tensor.matmul(
                    sc[:, 0:S], qoT[pt][:, 128 * tt : 128 * (tt + 1)], kfT[pt][:],
                    start=(pt == 0), stop=(pt == 1),
                )
            rowmax = pc.tile([128, 1], F32, tag="rmax2")
            nc.vector.tensor_reduce(
                rowmax[:], sc[:, 0:S], axis=mybir.AxisListType.X,
                op=mybir.AluOpType.max,
            )
            nb = pc.tile([128, 1], F32, tag="nb2")
            nc.vector.tensor_scalar_mul(nb[:], rowmax[:], -SCALE)
            a2 = pc.tile([128, S], F32, tag="a2")
            sumexp = pc.tile([128, 1], F32, tag="sexp2")
            nc.scalar.activation(
                a2[:], sc[:, 0:S], mybir.ActivationFunctionType.Exp,
                bias=nb[:], scale=SCALE, accum_out=sumexp[:],
            )
            rs = pc.tile([128, 1], F32, tag="rs2")
            nc.vector.reciprocal(rs[:], sumexp[:])
            nc.vector.tensor_scalar_mul(a2[:], a2[:], rs[:])
            ptr = cps.tile([128, L], F32, tag="c")
            nc.tensor.transpose(ptr[:, 0:S], a2[:], ident[:])
            nc.vector.tensor_copy(a2T[:, 128 * tt : 128 * (tt + 1)], ptr[:, 0:S])

        # Y^T [d128 x2, t512]
        yT = [pc.tile([128, L], F32, name=f"yT{dt}") for dt in range(2)]
        for dt in range(2):
            ps = cps.tile([128, L], F32, tag="c")
            nc.tensor.matmul(
                ps[:], vf[:, 128 * dt : 128 * (dt + 1)], a2T[:],
                start=True, stop=True,
            )
            nc.vector.tensor_copy(yT[dt][:], ps[:])

        nc.sync.dma_start(io["dbg"][:, 1536:2048], yT[0][:])
        # logits tiles + direct PSUM->DRAM store
        for vt in range(NVT):
            wo_sb = wop.tile([128, 2, VT], F32, tag="wo")
            nc.sync.dma_start(
                wo_sb[:], io["woutT"][vt].rearrange("dt dp v -> dp dt v")
            )
            for tt in range(4):
                lg = lgps.tile([128, VT], F32, tag="lg")
                for dt in range(2):
                    nc.tensor.matmul(
                        lg[:],
                        _mm(yT[dt][:, 128 * tt : 128 * (tt + 1)]),
                        _mm(wo_sb[:, dt, :]),
                        start=(dt == 0), stop=(dt == 1),
                    )
                lg_sb = wop.tile([128, VT], F32, tag="lg_sb", name="lg_sb")
                nc.any.tensor_copy(lg_sb[:], lg[:])
                nc.sync.dma_start(
                    io["lg_out"][
                        128 * tt : 128 * (tt + 1), VT * vt : VT * (vt + 1)
                    ],
                    lg_sb[:],
                )


# ---------------------------------------------------------------------------
# Host side
# ---------------------------------------------------------------------------

_NC_CACHE = {}


def _get_nc():
    key = (MM_MODE, N_STEPS)
    if key not in _NC_CACHE:
        _NC_CACHE[key] = build()
    return _NC_CACHE[key]


def _prep_in_maps(inputs):
    f32 = lambda a: np.ascontiguousarray(np.asarray(a), dtype=np.float32)
    input_ids = np.asarray(inputs["input_ids"])
    attention_mask = np.asarray(inputs["attention_mask"])
    H = f32(inputs["H"])
    W_source = f32(inputs["W_source"])
    W_target = f32(inputs["W_target"])

    lngb = np.zeros((B, STEPS, 2 * D), dtype=np.float32)
    lngb[:, :, 0:D] = np.asarray(inputs["ln_scale"])[None]
    lngb[:, :, D:] = np.asarray(inputs["ln_bias"])[None]

    rep = {
        "wqT": f32(np.asarray(inputs["Wq_in"]).T),
        "wkslT": f32(np.asarray(inputs["Wk_slots"]).T),
        "wvT": f32(np.asarray(inputs["Wv_in"]).T),
        "wqoT": f32(np.asarray(inputs["Wq_out"]).T),
        "wkfT": f32(np.asarray(inputs["Wk_fin"]).T),
        "wvfT": f32(np.asarray(inputs["Wv_fin"]).T),
        "hT_in": f32(H.T),
        "hn_in": H,
        "lngb": lngb,
        # woutT[vt, dtile, dp, vl] = Wout[500vt+vl, 128dt+dp]
        "woutT": np.ascontiguousarray(
            f32(inputs["W_out_proj"]).reshape(NVT, VT, 2, 128).transpose(0, 2, 3, 1)
        ),
    }

    in_maps = []
    for c in range(NC):
        m = dict(rep)
        X = (np.asarray(inputs["token_emb"], dtype=np.float32)[input_ids[c]]
             + np.asarray(inputs["pos_emb"], dtype=np.float32))
        m["xT_in"] = np.ascontiguousarray(X.T)
        m["maskw"] = np.ascontiguousarray(
            attention_mask[c].astype(np.float32).reshape(4, 128).T
        )
        # wsrc[i, k, dt, dp, (jloc r)] = W_source[i, 16c+4k+jloc, 128dt+dp, r]
        ws = W_source[:, JL * c : JL * (c + 1)]      # [S, 16, D, R]
        ws = ws.reshape(S, K, 4, 2, 128, R).transpose(0, 1, 3, 4, 2, 5)
        m["wsrc"] = np.ascontiguousarray(ws).reshape(S, K, 2, 128, 4 * R)
        # wtgt[k, g, (jloc r), il, d] = W_target[4g+il, 16c+4k+jloc, r, d]
        # The reference masks out the i == j (diagonal) pair; zeroing
        # W_target[j, j] is exactly equivalent since the term is linear in it.
        wt = W_target[:, JL * c : JL * (c + 1)].copy()   # [S, 16, R, D]
        for jl in range(JL):
            wt[JL * c + jl, jl] = 0.0
        wt = wt.reshape(32, 4, K, 4, R, D).transpose(2, 0, 3, 4, 1, 5)
        m["wtgt"] = np.ascontiguousarray(wt).reshape(K, 32, 4 * R, 4, D)
        in_maps.append(m)
    return in_maps


def run(inputs, trace=False):
    nc = _get_nc()
    in_maps = _prep_in_maps(inputs)
    res = bass_utils.run_bass_kernel_spmd(
        nc, in_maps, core_ids=list(range(NC)), trace=trace
    )
    out = np.stack([res.results[c]["lg_out"] for c in range(NC)], axis=0)
    return out, res


def kernel(**inputs):
    out, _ = run(inputs, trace=False)
    return out



# revision 43
# speedup vs baseline: 34.3238x; 34.3238x over previous
"""Trainium2 Bass kernel for nn_ConnectionTransformer (8 NeuronCores, SPMD).

Strategy
--------
- Phase A (embed + compress attention): batch-parallel, core c handles batch c.
- Phase B (6 bilinear message-passing steps): target-slot sharding — core c owns
  16 target slots j in [16c, 16c+16). Each core computes the full influence for
  its slots (sum over all source slots i), applies relu/residual/LayerNorm
  locally, and an AllGather rebuilds the replicated transposed state h^T each
  step. The per-pair weights W_source/W_target are cast to bf16 on the host
  (512 MB total), sharded along j and streamed from HBM once per step per core
  (32+32 MB) in large contiguous DMAs — the memory roofline.
- Phase C (expand attention + vocab projection): batch-parallel again; W_out in
  bf16, logits emitted as bf16 and upcast on the host.

All weights are pre-transposed/tiled on the host into matmul-ready layouts so
the device never transposes weight tensors. All big matmuls run in bf16
(full-rate PE, half DMA traffic); LayerNorm/softmax accumulate in fp32.
"""
import os
import sys

sys.path.insert(0, "/opt/trn_rl_repo")

import numpy as np
import ml_dtypes
from concourse import bass, bacc, tile, bass_utils, mybir
from concourse import masks

B, L, D, S, R, STEPS, V = 8, 512, 256, 128, 32, 6, 32000
NC = 8
JL = S // NC          # 16 local target slots per core
K = JL // 4           # 4 quads of target slots
VT = 500              # vocab tile width
NVT = V // VT         # 64 vocab tiles
SCALE = 1.0 / np.sqrt(D)
LN_EPS = 1e-5

F32 = mybir.dt.float32
BF16 = mybir.dt.bfloat16
NP_BF16 = ml_dtypes.bfloat16

N_STEPS = int(os.environ.get("N_STEPS", str(STEPS)))
# Q: einsum2 contracts 32 (r) with 4-way row tile_position packing.
# P: einsum1 scatters to (il,r) partitions via column tile_position;
#    einsum2 contracts 128 in 4x fewer matmuls. NOTE: broken on HW — the
#    il=3 column tile needs PE column quadrant 3, which cannot take weight
#    loads (known HW limitation), so results are silently wrong. Kept for
#    cost-model experiments only.
LAYOUT = os.environ.get("LAYOUT", "Q")


# ---------------------------------------------------------------------------
# Device program
# ---------------------------------------------------------------------------

def build():
    nc = bacc.Bacc("TRN2", target_bir_lowering=False, debug=False, num_devices=NC)

    io = {}

    def inp(name, shape, dtype=F32):
        io[name] = nc.dram_tensor(name, shape, dtype, kind="ExternalInput").ap()

    inp("xT_in", [D, L])
    inp("maskw", [128, 4])
    for w in ("wqT", "wkslT", "wvT", "wqoT", "wkfT", "wvfT"):
        inp(w, [D, D])
    inp("hT_in", [D, S])
    inp("hn_in", [S, D])
    inp("lngT", [128, STEPS, 2, 2])
    if LAYOUT == "Q":
        # ws[ig, dp, (i8 q4 dt2 jl4 r32)] : per-ig chunk of 8 source slots
        inp("wsrc", [16, 128, 8 * K * 2 * 128], BF16)
        # wt[q, gg, (jl r)=128, (gl4 il4 d256)]
        inp("wtgt", [K, 8, 128, 4 * 4 * D], BF16)
    else:
        # ws[ig, dp, (il4 j16 dt2 r32)] : per-ig chunk of 4 source slots
        inp("wsrc", [32, 128, 4 * JL * 2 * R], BF16)
        # wt[q, gg, (il r)=128, (igl4 jl4 d256)]
        inp("wtgt", [K, 8, 128, 4 * 4 * D], BF16)
    inp("woutT", [NVT, 128, 2 * VT], BF16)
    io["lg_out"] = nc.dram_tensor(
        "lg_out", [L, V], BF16, kind="ExternalOutput"
    ).ap()
    io["dbg"] = nc.dram_tensor(
        "dbg", [128, 2048], F32, kind="ExternalOutput"
    ).ap()
    with tile.TileContext(nc) as tc:
        _body(nc, tc, io)
    nc.compile()
    return nc


def _body(nc, tc, io):
    with tc.tile_pool(name="const", bufs=1) as const, \
         tc.tile_pool(name="state", bufs=1) as state:

        ident = const.tile([128, 128], F32)
        masks.make_identity(nc, ident[:])
        ones = const.tile([128, 1], F32)
        nc.vector.memset(ones[:], 1.0)
        ones_row = const.tile([1, 128], F32)
        nc.vector.memset(ones_row[:], 1.0)
        eps_sb = const.tile([128, 1], F32)
        nc.vector.memset(eps_sb[:], LN_EPS)

        pid = nc.sync.partition_id()

        # persistent state
        hT16 = [state.tile([128, S, B], BF16, name=f"hT{dt}") for dt in range(2)]
        # own 16 slots, transposed: [dp, dt, jl, b]
        hown32 = state.tile([128, 2, JL, B], F32)
        hown16 = state.tile([128, 2, JL, B], BF16)
        qoT = [state.tile([128, L], F32, name=f"qoT{pt}") for pt in range(2)]
        lngT_sb = state.tile([128, STEPS, 2, 2], F32)
        nc.sync.dma_start(lngT_sb[:], io["lngT"])

        _phase_a(nc, tc, io, ident, ones, pid, hT16, hown32, hown16, qoT)
        nc.sync.dma_start(
            io["dbg"][:, 0:256],
            hown32[:].rearrange("p dt jl b -> p (dt jl b)"),
        )
        for t in range(N_STEPS):
            _step(nc, tc, t, io["wsrc"], io["wtgt"], hT16, hown32, hown16,
                  lngT_sb, ident, ones, ones_row, eps_sb,
                  io["dbg"] if t == 0 else None)
        _phase_c(nc, tc, io, ident, pid, hT16, qoT)


def _phase_a(nc, tc, io, ident, ones, pid, hT16, hown32, hown16, qoT):
    with tc.tile_pool(name="pa_sb", bufs=1) as pa, \
         tc.tile_pool(name="pa_ps", bufs=3, space="PSUM") as pps, \
         tc.tile_pool(name="pa_tp", bufs=2, space="PSUM") as tps, \
         tc.tile_pool(name="pa_acc", bufs=1, space="PSUM") as aps, \
         tc.tile_pool(name="dram_a", bufs=1, space="DRAM") as dra:

        mask_sb = pa.tile([128, 4], F32)
        nc.sync.dma_start(mask_sb[:], io["maskw"])

        # X^T tiles [d128, t512] (host-gathered embeddings, transposed)
        xT = [pa.tile([128, L], F32, name=f"xT{ct}") for ct in range(2)]
        for ct in range(2):
            nc.sync.dma_start(xT[ct][:], io["xT_in"][128 * ct : 128 * (ct + 1), :])

        # weight tiles [d128, 256] (contraction on partitions)
        def load_w(name):
            ts = [pa.tile([128, D], F32, name=f"{name}_{ct}") for ct in range(2)]
            for ct in range(2):
                nc.sync.dma_start(ts[ct][:], io[name][128 * ct : 128 * (ct + 1), :])
            return ts

        wq_sb = load_w("wqT")
        wv_sb = load_w("wvT")
        wksl_sb = load_w("wkslT")
        wqo_sb = load_w("wqoT")
        hTt = [pa.tile([128, S], F32, name=f"hTt{ct}") for ct in range(2)]
        for ct in range(2):
            nc.sync.dma_start(hTt[ct][:], io["hT_in"][128 * ct : 128 * (ct + 1), :])
        hn_sb = pa.tile([S, D], F32)
        nc.sync.dma_start(hn_sb[:], io["hn_in"])

        # Q_in^T and Q_out^T : [d'128 x 2, t512]
        qT = [pa.tile([128, L], F32, name=f"qT{pt}") for pt in range(2)]
        for pt in range(2):
            for dst, wsb in ((qT, wq_sb), (qoT, wqo_sb)):
                ps = pps.tile([128, L], F32, tag="ps")
                for ct in range(2):
                    nc.tensor.matmul(
                        ps[:], wsb[ct][:, 128 * pt : 128 * (pt + 1)], xT[ct][:],
                        start=(ct == 0), stop=(ct == 1),
                    )
                nc.vector.tensor_copy(dst[pt][:], ps[:])

        # V_in natural [t128 x 4, d256]
        vn = pa.tile([128, 4, D], F32)
        for tt in range(4):
            ps = pps.tile([128, L], F32, tag="ps")
            for ct in range(2):
                nc.tensor.matmul(
                    ps[:, 0:D], xT[ct][:, 128 * tt : 128 * (tt + 1)], wv_sb[ct][:],
                    start=(ct == 0), stop=(ct == 1),
                )
            nc.vector.tensor_copy(vn[:, tt, :], ps[:, 0:D])

        # K_slots^T [d'128 x 2, s128]
        kslT = [pa.tile([128, S], F32, name=f"kslT{pt}") for pt in range(2)]
        for pt in range(2):
            ps = pps.tile([128, L], F32, tag="ps")
            for ct in range(2):
                nc.tensor.matmul(
                    ps[:, 0:S], wksl_sb[ct][:, 128 * pt : 128 * (pt + 1)], hTt[ct][:],
                    start=(ct == 0), stop=(ct == 1),
                )
            nc.vector.tensor_copy(kslT[pt][:], ps[:, 0:S])

        # attention scores + masked softmax
        a_sb = pa.tile([128, 4, S], F32)
        for tt in range(4):
            sc = pps.tile([128, L], F32, tag="ps")
            for pt in range(2):
                nc.tensor.matmul(
                    sc[:, 0:S], qT[pt][:, 128 * tt : 128 * (tt + 1)], kslT[pt][:],
                    start=(pt == 0), stop=(pt == 1),
                )
            rowmax = pa.tile([128, 1], F32, tag="rmax")
            nc.vector.tensor_reduce(
                rowmax[:], sc[:, 0:S], axis=mybir.AxisListType.X,
                op=mybir.AluOpType.max,
            )
            nb = pa.tile([128, 1], F32, tag="nb")
            nc.vector.tensor_scalar_mul(nb[:], rowmax[:], -SCALE)
            sumexp = pa.tile([128, 1], F32, tag="sexp")
            nc.scalar.activation(
                a_sb[:, tt, :], sc[:, 0:S], mybir.ActivationFunctionType.Exp,
                bias=nb[:], scale=SCALE, accum_out=sumexp[:],
            )
            rs = pa.tile([128, 1], F32, tag="rs")
            nc.vector.reciprocal(rs[:], sumexp[:])
            rm = pa.tile([128, 1], F32, tag="rmk")
            nc.vector.tensor_tensor(
                rm[:], rs[:], mask_sb[:, tt : tt + 1], op=mybir.AluOpType.mult
            )
            nc.vector.tensor_scalar_mul(a_sb[:, tt, :], a_sb[:, tt, :], rm[:])

        # column sums and IR = A^T @ V
        cs = aps.tile([128, 1], F32, tag="cs")
        for tt in range(4):
            nc.tensor.matmul(
                cs[:], a_sb[:, tt, :], ones[:, 0:1], start=(tt == 0), stop=(tt == 3)
            )
        ir = aps.tile([128, D], F32, tag="ir")
        for tt in range(4):
            nc.tensor.matmul(
                ir[:], a_sb[:, tt, :], vn[:, tt, :], start=(tt == 0), stop=(tt == 3)
            )
        cssb = pa.tile([128, 1], F32)
        nc.vector.tensor_scalar_add(cssb[:], cs[:], 1e-8)
        rcs = pa.tile([128, 1], F32)
        nc.vector.reciprocal(rcs[:], cssb[:])
        h0 = pa.tile([S, D], F32)
        nc.vector.scalar_tensor_tensor(
            h0[:], ir[:], rcs[:], hn_sb[:],
            op0=mybir.AluOpType.mult, op1=mybir.AluOpType.add,
        )

        # h0 -> transposed bounce (bf16), AllGather
        agin0 = dra.tile([2 * 16384], BF16)
        for dt in range(2):
            p3 = tps.tile([128, 128], F32, tag="tp")
            nc.tensor.transpose(p3[:], h0[:, 128 * dt : 128 * (dt + 1)], ident[:])
            h0T = pa.tile([128, 128], BF16, tag="h0T")
            nc.vector.tensor_copy(h0T[:], p3[:])
            nc.sync.dma_start(
                agin0[dt * 16384 : (dt + 1) * 16384].rearrange(
                    "(p f) -> p f", p=128
                ),
                h0T[:],
            )
        agout0 = dra.tile([NC, 2 * 16384], BF16, addr_space="Shared")
        nc.gpsimd.collective_compute(
            "AllGather", mybir.AluOpType.bypass,
            ins=[agin0[:].opt()], outs=[agout0[:].opt()],
            replica_groups=[list(range(NC))],
        )
        # readback: hT16[dt][dp, s, b] ; hown16/32 (own transposed slice via pid)
        ag0r = agout0[:].rearrange(
            "b (seg dp s) -> seg dp s b", seg=2, dp=128, s=128
        )
        pid_v = nc.vector.partition_id()
        for dt in range(2):
            nc.sync.dma_start(hT16[dt][:], ag0r[dt])
            nc.vector.tensor_copy(
                hown16[:, dt, :, :].rearrange("p jl b -> p (jl b)"),
                hT16[dt][:].rearrange("p s b -> p (s b)")[
                    :, bass.ds(pid_v * (JL * B), JL * B)
                ],
            )
        nc.vector.tensor_copy(hown32[:], hown16[:])


def _step(nc, tc, t, wsrc, wtgt, hT16, hown32, hown16, lngT_sb, ident, ones,
          ones_row, eps_sb, dbg=None):
    """One message-passing step."""
    with tc.tile_pool(name=f"s{t}_ws", bufs=3) as wsp, \
         tc.tile_pool(name=f"s{t}_wt", bufs=3) as wtp, \
         tc.tile_pool(name=f"s{t}_sb", bufs=1) as sb, \
         tc.tile_pool(name=f"s{t}_p1", bufs=1, space="PSUM") as p1p, \
         tc.tile_pool(name=f"s{t}_p2", bufs=1, space="PSUM") as p2p, \
         tc.tile_pool(name=f"s{t}_p3", bufs=1, space="PSUM") as p3p, \
         tc.tile_pool(name=f"s{t}_p4", bufs=1, space="PSUM") as p4p, \
         tc.tile_pool(name=f"s{t}_dram", bufs=1, space="DRAM") as drp:

        if LAYOUT == "Q":
            inter = sb.tile([128, K, S, B], BF16)
            # ---- einsum1: inter[(jl,r), q, i, b] = h[b,i,:] @ Ws[i, j] ----
            for ig in range(16):
                ws = wsp.tile([128, 8, K, 2, 128], BF16, tag="ws")
                nc.sync.dma_start(ws[:], wsrc[ig].rearrange("dp (i q dt jr) -> dp i q dt jr", i=8, q=K, dt=2))
                for i8 in range(8):
                    i = 8 * ig + i8
                    p1 = p1p.tile([128, K, B], F32, tag="p1")
                    for q in range(K):
                        for dt in range(2):
                            nc.tensor.matmul(
                                p1[:, q, :],
                                ws[:, i8, q, dt, :],
                                hT16[dt][:, i, :],
                                start=(dt == 0), stop=(dt == 1),
                            )
                    nc.vector.tensor_copy(inter[:, :, i, :], p1[:])
        else:
            # ---- einsum1: inter2[(il,r), ig, j, b] via column tile_position ----
            inter2 = sb.tile([128, 32, JL, B], BF16)
            for ig in range(32):
                ws = wsp.tile([128, 4, JL, 2, R], BF16, tag="ws")
                nc.sync.dma_start(ws[:], wsrc[ig].rearrange("dp (il j dt r) -> dp il j dt r", il=4, j=JL, dt=2))
                p1 = p1p.tile([128, JL, B], F32, tag="p1")
                for il in range(4):
                    i = 4 * ig + il
                    for j in range(JL):
                        for dt in range(2):
                            nc.tensor.matmul(
                                p1[32 * il : 32 * (il + 1), j, :],
                                ws[:, il, j, dt, :],
                                hT16[dt][:, i, :],
                                start=(dt == 0), stop=(dt == 1),
                                tile_position=(0, 32 * il),
                            )
                nc.vector.tensor_copy(inter2[:, ig, :, :], p1[:])

        # ---- einsum2 per quad q; relu + transpose into tcandT ----
        tcandT = sb.tile([128, 2, JL, B], F32)   # relu(infl)^T
        for q in range(K):
            # one PSUM bank per jl — concurrent row tiles must not share banks
            p2 = [p2p.tile([B, D], F32, tag=f"p2_{jl}", name=f"p2_{jl}")[:] for jl in range(4)]
            for gg in range(8):
                wt = wtp.tile([128, 4, 4, D], BF16, tag="wt")
                nc.sync.dma_start(wt[:], wtgt[q, gg].rearrange("p (gl il d) -> p gl il d", gl=4, il=4))
                if LAYOUT == "Q":
                    for gl in range(4):
                        for il in range(4):
                            i = 16 * gg + 4 * gl + il
                            for jl in range(4):
                                nc.tensor.matmul(
                                    p2[jl],
                                    inter[32 * jl : 32 * (jl + 1), q, i, :],
                                    wt[32 * jl : 32 * (jl + 1), gl, il, :],
                                    start=(i == 0), stop=(i == S - 1),
                                    tile_position=(32 * jl, 0),
                                )
                else:
                    for igl in range(4):
                        ig = 4 * gg + igl
                        for jl in range(4):
                            nc.tensor.matmul(
                                p2[jl],
                                inter2[:, ig, 4 * q + jl, :],
                                wt[:, igl, jl, :],
                                start=(ig == 0), stop=(ig == 31),
                            )
            hrelu = sb.tile([B, 4, D], F32, tag="hrelu")
            for jl in range(4):
                nc.scalar.activation(
                    hrelu[:, jl, :], p2[jl],
                    mybir.ActivationFunctionType.Relu,
                )
            for jloc in range(4):
                for dt in range(2):
                    p3 = p3p.tile([128, B], F32, tag="p3")
                    nc.tensor.transpose(
                        p3[:],
                        hrelu[:, jloc, 128 * dt : 128 * (dt + 1)],
                        ident[0:B, 0:B],
                    )
                    nc.vector.tensor_copy(
                        tcandT[:, dt, 4 * q + jloc, :], p3[:]
                    )

        if dbg is not None:
            nc.sync.dma_start(
                dbg[:, 256:512],
                tcandT[:].rearrange("p dt jl b -> p (dt jl b)"),
            )
        # ---- residual + LayerNorm in transposed layout (128 lanes) ----
        # hx holds x and x^2 side by side so ONE accumulation group computes
        # both partition sums (interleaved psum groups sharing a bank lose
        # the earlier group's partial on the later group's start).
        hx = sb.tile([128, 2, 2, JL, B], F32, tag="hx")  # [dp, dt, {x,x2}, jl, b]
        hcand = hx[:, :, 0, :, :]
        nc.vector.tensor_tensor(
            hcand, tcandT[:], hown32[:], op=mybir.AluOpType.add
        )
        if dbg is not None:
            nc.sync.dma_start(
                dbg[:, 512:768].rearrange("p (dt f) -> p dt f", dt=2),
                hcand.rearrange("p dt jl b -> p dt (jl b)"),
            )
        nc.vector.tensor_tensor(
            hx[:, :, 1, :, :], hcand, hcand, op=mybir.AluOpType.mult
        )
        s12 = p4p.tile([1, 2, JL * B], F32, tag="s12")
        for dt in range(2):
            nc.tensor.matmul(
                s12[:].rearrange("o c f -> o (c f)"), ones[:, 0:1],
                hx[:, dt, :, :, :].rearrange("p c jl b -> p (c jl b)"),
                start=(dt == 0), stop=(dt == 1),
            )
        mean = sb.tile([1, JL * B], F32, tag="mean")
        nc.vector.tensor_scalar_mul(mean[:], s12[:, 0, :], 1.0 / D)
        ex2 = sb.tile([1, JL * B], F32, tag="ex2")
        nc.vector.tensor_scalar_mul(ex2[:], s12[:, 1, :], 1.0 / D)
        m2 = sb.tile([1, JL * B], F32, tag="m2")
        nc.vector.tensor_tensor(m2[:], mean[:], mean[:], op=mybir.AluOpType.mult)
        var = sb.tile([1, JL * B], F32, tag="var")
        nc.vector.tensor_tensor(var[:], ex2[:], m2[:], op=mybir.AluOpType.subtract)
        std = sb.tile([1, JL * B], F32, tag="std")
        nc.scalar.activation(
            std[:], var[:], mybir.ActivationFunctionType.Sqrt,
            bias=eps_sb[0:1, :], scale=1.0,
        )
        # mean/rstd concat -> single broadcast matmul (one group)
        mr = sb.tile([1, 2, JL * B], F32, tag="mr")
        nc.vector.tensor_copy(mr[:, 0, :], mean[:])
        nc.vector.reciprocal(mr[:, 1, :], std[:])
        rstd = mr[:, 1, :]
        mbrb = p3p.tile([128, 2, JL * B], F32, tag="mbrb")
        nc.tensor.matmul(
            mbrb[:].rearrange("p c f -> p (c f)"), ones_row[:],
            mr[:].rearrange("o c f -> o (c f)"),
            start=True, stop=True,
        )
        t1 = sb.tile([128, 2, JL, B], F32, tag="t1")
        mb_r = mbrb[:, 0, :].rearrange("p (jl b) -> p jl b", jl=JL)
        rb_r = mbrb[:, 1, :].rearrange("p (jl b) -> p jl b", jl=JL)
        for dt in range(2):
            nc.vector.tensor_tensor(
                t1[:, dt, :, :], hcand[:, dt, :, :], mb_r,
                op=mybir.AluOpType.subtract,
            )
            nc.vector.tensor_tensor(
                t1[:, dt, :, :], t1[:, dt, :, :], rb_r,
                op=mybir.AluOpType.mult,
            )
            nc.vector.tensor_scalar(
                hown32[:, dt, :, :], t1[:, dt, :, :],
                lngT_sb[:, t, dt, 0:1], lngT_sb[:, t, dt, 1:2],
                op0=mybir.AluOpType.mult, op1=mybir.AluOpType.add,
            )
        nc.vector.tensor_copy(hown16[:], hown32[:])
        if dbg is not None:
            nc.sync.dma_start(
                dbg[:, 1024:1280],
                hown32[:].rearrange("p dt jl b -> p (dt jl b)"),
            )
            nc.sync.dma_start(dbg[0:1, 768:896], mean[:])
            nc.sync.dma_start(dbg[0:1, 896:1024], rstd)

        # ---- AllGather the own transposed slots (bf16); rebuild hT16 ----
        agin = drp.tile([2 * 128 * JL * B], BF16)
        for dt in range(2):
            nc.sync.dma_start(
                agin[dt * 16384 : (dt + 1) * 16384].rearrange(
                    "(p f) -> p f", p=128
                ),
                hown16[:, dt, :, :],
            )
        agout = drp.tile([NC, 2 * 128 * JL * B], BF16, addr_space="Shared")
        nc.gpsimd.collective_compute(
            "AllGather", mybir.AluOpType.bypass,
            ins=[agin[:].opt()], outs=[agout[:].opt()],
            replica_groups=[list(range(NC))],
        )
        agr = agout[:].rearrange(
            "rk (dt dp jl b) -> dt dp rk jl b", dt=2, dp=128, jl=JL, b=B
        )
        for dt in range(2):
            nc.sync.dma_start(
                hT16[dt][:].rearrange("dp (rk jl) b -> dp rk jl b", rk=NC), agr[dt]
            )


def _phase_c(nc, tc, io, ident, pid, hT16, qoT):
    with tc.tile_pool(name="pc_sb", bufs=1) as pc, \
         tc.tile_pool(name="pc_ps", bufs=3, space="PSUM") as cps, \
         tc.tile_pool(name="pc_lg", bufs=4, space="PSUM") as lgps, \
         tc.tile_pool(name="pc_wo", bufs=4) as wop:

        wkf_sb = [pc.tile([128, D], F32, name=f"wkf{ct}") for ct in range(2)]
        wvf_sb = [pc.tile([128, D], F32, name=f"wvf{ct}") for ct in range(2)]
        for ct in range(2):
            nc.sync.dma_start(
                wkf_sb[ct][:], io["wkfT"][128 * ct : 128 * (ct + 1), :]
            )
            nc.sync.dma_start(
                wvf_sb[ct][:], io["wvfT"][128 * ct : 128 * (ct + 1), :]
            )

        # own-batch h^T slice (dynamic b=pid) -> static f32 tiles
        pid_v = nc.vector.partition_id()
        hb = [pc.tile([128, S], F32, name=f"hb{dt}") for dt in range(2)]
        for dt in range(2):
            nc.vector.tensor_copy(
                hb[dt][:].rearrange("p (s o) -> p s o", o=1),
                hT16[dt][:, :, bass.ds(pid_v, 1)],
            )

        # K_f^T [d'128 x2, s128] ; V_f natural [s, d']
        kfT = [pc.tile([128, S], F32, name=f"kfT{pt}") for pt in range(2)]
        for pt in range(2):
            ps = cps.tile([128, L], F32, tag="c")
            for ct in range(2):
                nc.tensor.matmul(
                    ps[:, 0:S], wkf_sb[ct][:, 128 * pt : 128 * (pt + 1)], hb[ct][:],
                    start=(ct == 0), stop=(ct == 1),
                )
            nc.vector.tensor_copy(kfT[pt][:], ps[:, 0:S])
        vf = pc.tile([S, D], F32)
        psv = cps.tile([128, L], F32, tag="c")
        for ct in range(2):
            nc.tensor.matmul(
                psv[0:S, 0:D], hb[ct][:], wvf_sb[ct][:],
                start=(ct == 0), stop=(ct == 1),
            )
        nc.vector.tensor_copy(vf[:], psv[0:S, 0:D])

        # expand attention -> A2^T [s, t512]
        a2T = pc.tile([S, L], F32)
        for tt in range(4):
            sc = cps.tile([128, L], F32, tag="c")
            for pt in range(2):
                nc.tensor.matmul(
                    sc[:, 0:S], qoT[pt][:, 128 * tt : 128 * (tt + 1)], kfT[pt][:],
                    start=(pt == 0), stop=(pt == 1),
                )
            rowmax = pc.tile([128, 1], F32, tag="rmax2")
            nc.vector.tensor_reduce(
                rowmax[:], sc[:, 0:S], axis=mybir.AxisListType.X,
                op=mybir.AluOpType.max,
            )
            nb = pc.tile([128, 1], F32, tag="nb2")
            nc.vector.tensor_scalar_mul(nb[:], rowmax[:], -SCALE)
            a2 = pc.tile([128, S], F32, tag="a2")
            sumexp = pc.tile([128, 1], F32, tag="sexp2")
            nc.scalar.activation(
                a2[:], sc[:, 0:S], mybir.ActivationFunctionType.Exp,
                bias=nb[:], scale=SCALE, accum_out=sumexp[:],
            )
            rs = pc.tile([128, 1], F32, tag="rs2")
            nc.vector.reciprocal(rs[:], sumexp[:])
            nc.vector.tensor_scalar_mul(a2[:], a2[:], rs[:])
            ptr = cps.tile([128, L], F32, tag="c")
            nc.tensor.transpose(ptr[:, 0:S], a2[:], ident[:])
            nc.vector.tensor_copy(a2T[:, 128 * tt : 128 * (tt + 1)], ptr[:, 0:S])

        # Y^T [d128 x2, t512] -> bf16
        yT16 = [pc.tile([128, L], BF16, name=f"yT{dt}") for dt in range(2)]
        for dt in range(2):
            ps = cps.tile([128, L], F32, tag="c")
            nc.tensor.matmul(
                ps[:], vf[:, 128 * dt : 128 * (dt + 1)], a2T[:],
                start=True, stop=True,
            )
            nc.vector.tensor_copy(yT16[dt][:], ps[:])

        # logits tiles (bf16 matmul) + bf16 store
        for vt in range(NVT):
            wo_sb = wop.tile([128, 2, VT], BF16, tag="wo")
            nc.sync.dma_start(
                wo_sb[:], io["woutT"][vt].rearrange("dp (dt v) -> dp dt v", dt=2)
            )
            for tt in range(4):
                lg = lgps.tile([128, VT], F32, tag="lg")
                for dt in range(2):
                    nc.tensor.matmul(
                        lg[:],
                        yT16[dt][:, 128 * tt : 128 * (tt + 1)],
                        wo_sb[:, dt, :],
                        start=(dt == 0), stop=(dt == 1),
                    )
                lg_sb = wop.tile([128, VT], BF16, tag="lg_sb", name="lg_sb")
                nc.any.tensor_copy(lg_sb[:], lg[:])
                nc.sync.dma_start(
                    io["lg_out"][
                        128 * tt : 128 * (tt + 1), VT * vt : VT * (vt + 1)
                    ],
                    lg_sb[:],
                )


# ---------------------------------------------------------------------------
# Host side
# ---------------------------------------------------------------------------

_NC_CACHE = {}


def _get_nc():
    key = (N_STEPS, LAYOUT)
    if key not in _NC_CACHE:
        _NC_CACHE[key] = build()
    return _NC_CACHE[key]


def _prep_in_maps(inputs):
    f32 = lambda a: np.ascontiguousarray(np.asarray(a), dtype=np.float32)
    input_ids = np.asarray(inputs["input_ids"])
    attention_mask = np.asarray(inputs["attention_mask"])
    H = f32(inputs["H"])
    W_source = f32(inputs["W_source"])
    W_target = f32(inputs["W_target"])

    # lngT[dp, t, dt, {scale, bias}] = ln_{scale,bias}[t, 128dt+dp]
    lngT = np.stack(
        [
            f32(inputs["ln_scale"]).reshape(STEPS, 2, 128).transpose(2, 0, 1),
            f32(inputs["ln_bias"]).reshape(STEPS, 2, 128).transpose(2, 0, 1),
        ],
        axis=-1,
    )

    rep = {
        "wqT": f32(np.asarray(inputs["Wq_in"]).T),
        "wkslT": f32(np.asarray(inputs["Wk_slots"]).T),
        "wvT": f32(np.asarray(inputs["Wv_in"]).T),
        "wqoT": f32(np.asarray(inputs["Wq_out"]).T),
        "wkfT": f32(np.asarray(inputs["Wk_fin"]).T),
        "wvfT": f32(np.asarray(inputs["Wv_fin"]).T),
        "hT_in": f32(H.T),
        "hn_in": H,
        "lngT": np.ascontiguousarray(lngT),
        # woutT[vt, dp, (dt, vl)] = Wout[500vt+vl, 128dt+dp]  (bf16)
        "woutT": np.ascontiguousarray(
            f32(inputs["W_out_proj"]).reshape(NVT, VT, 2, 128)
            .transpose(0, 3, 2, 1).reshape(NVT, 128, 2 * VT)
        ).astype(NP_BF16),
    }

    in_maps = []
    for c in range(NC):
        m = dict(rep)
        X = (np.asarray(inputs["token_emb"], dtype=np.float32)[input_ids[c]]
             + np.asarray(inputs["pos_emb"], dtype=np.float32))
        m["xT_in"] = np.ascontiguousarray(X.T)
        m["maskw"] = np.ascontiguousarray(
            attention_mask[c].astype(np.float32).reshape(4, 128).T
        )
        ws = W_source[:, JL * c : JL * (c + 1)]      # [S, 16, D, R]
        # The reference masks out the i == j (diagonal) pair; zeroing
        # W_target[j, j] is exactly equivalent since the term is linear in it.
        wt = W_target[:, JL * c : JL * (c + 1)].copy()   # [S, 16, R, D]
        for jl in range(JL):
            wt[JL * c + jl, jl] = 0.0
        if LAYOUT == "Q":
            # ws[ig, dp, (i8, q, dt, jl, r)] = Ws[8ig+i8, 16c+4q+jl, 128dt+dp, r]
            ws = ws.reshape(16, 8, K, 4, 2, 128, R).transpose(0, 5, 1, 2, 4, 3, 6)
            m["wsrc"] = np.ascontiguousarray(ws).reshape(
                16, 128, 8 * K * 2 * 128
            ).astype(NP_BF16)
            # wt[q, gg, (jl, r), (gl, il, d)] = Wt[16gg+4gl+il, 16c+4q+jl, r, d]
            wt = wt.reshape(8, 4, 4, K, 4, R, D).transpose(3, 0, 4, 5, 1, 2, 6)
            m["wtgt"] = np.ascontiguousarray(wt).reshape(
                K, 8, 128, 4 * 4 * D
            ).astype(NP_BF16)
        else:
            # ws[ig, dp, (il, j, dt, r)] = Ws[4ig+il, 16c+j, 128dt+dp, r]
            ws = ws.reshape(32, 4, JL, 2, 128, R).transpose(0, 4, 1, 2, 3, 5)
            m["wsrc"] = np.ascontiguousarray(ws).reshape(
                32, 128, 4 * JL * 2 * R
            ).astype(NP_BF16)
            # wt[q, gg, (il, r), (igl, jl, d)] = Wt[16gg+4igl+il, 16c+4q+jl, r, d]
            wt = wt.reshape(8, 4, 4, K, 4, R, D).transpose(3, 0, 2, 5, 1, 4, 6)
            m["wtgt"] = np.ascontiguousarray(wt).reshape(
                K, 8, 128, 4 * 4 * D
            ).astype(NP_BF16)
        in_maps.append(m)
    return in_maps


def run(inputs, trace=False):
    nc = _get_nc()
    in_maps = _prep_in_maps(inputs)
    res = bass_utils.run_bass_kernel_spmd(
        nc, in_maps, core_ids=list(range(NC)), trace=trace
    )
    out = np.stack(
        [res.results[c]["lg_out"].astype(np.float32) for c in range(NC)], axis=0
    )
    return out, res


def kernel(**inputs):
    out, _ = run(inputs, trace=False)
    return out


# revision 45
# speedup vs baseline: 38.0551x; 1.1087x over previous
"""Trainium2 Bass kernel for nn_ConnectionTransformer (8 NeuronCores, SPMD).

Strategy
--------
- Phase A (embed + compress attention): batch-parallel, core c handles batch c.
- Phase B (6 bilinear message-passing steps): target-slot sharding — core c owns
  16 target slots j in [16c, 16c+16). Each core computes the full influence for
  its slots (sum over all source slots i), applies relu/residual/LayerNorm
  locally, and an AllGather rebuilds the replicated transposed state h^T each
  step. The per-pair weights W_source/W_target are cast to bf16 on the host
  (512 MB total), sharded along j and streamed from HBM once per step per core
  (32+32 MB) in large contiguous DMAs — the memory roofline.
- Phase C (expand attention + vocab projection): batch-parallel again; W_out in
  bf16, logits emitted as bf16 and upcast on the host.

All weights are pre-transposed/tiled on the host into matmul-ready layouts so
the device never transposes weight tensors. All big matmuls run in bf16
(full-rate PE, half DMA traffic); LayerNorm/softmax accumulate in fp32.
"""
import os
import sys

sys.path.insert(0, "/opt/trn_rl_repo")

import numpy as np
import ml_dtypes
from concourse import bass, bacc, tile, bass_utils, mybir
from concourse import masks

B, L, D, S, R, STEPS, V = 8, 512, 256, 128, 32, 6, 32000
NC = 8
JL = S // NC          # 16 local target slots per core
K = JL // 4           # 4 quads of target slots
VT = 500              # vocab tile width
NVT = V // VT         # 64 vocab tiles
SCALE = 1.0 / np.sqrt(D)
LN_EPS = 1e-5

F32 = mybir.dt.float32
BF16 = mybir.dt.bfloat16
NP_BF16 = ml_dtypes.bfloat16

N_STEPS = int(os.environ.get("N_STEPS", str(STEPS)))
# Q: einsum2 contracts 32 (r) with 4-way row tile_position packing.
# P: einsum1 scatters to (il,r) partitions via column tile_position;
#    einsum2 contracts 128 in 4x fewer matmuls. NOTE: broken on HW — the
#    il=3 column tile needs PE column quadrant 3, which cannot take weight
#    loads (known HW limitation), so results are silently wrong. Kept for
#    cost-model experiments only.
LAYOUT = os.environ.get("LAYOUT", "Q")


# ---------------------------------------------------------------------------
# Device program
# ---------------------------------------------------------------------------

def build():
    nc = bacc.Bacc("TRN2", target_bir_lowering=False, debug=False, num_devices=NC)

    io = {}

    def inp(name, shape, dtype=F32):
        io[name] = nc.dram_tensor(name, shape, dtype, kind="ExternalInput").ap()

    inp("xT_in", [D, L])
    inp("maskw", [128, 4])
    for w in ("wqT", "wkslT", "wvT", "wqoT", "wkfT", "wvfT"):
        inp(w, [D, D])
    inp("hT_in", [D, S])
    inp("hn_in", [S, D])
    inp("lngT", [128, STEPS, 2, 2])
    if LAYOUT == "Q":
        # ws[ig, dp, (i8 q4 dt2 jl4 r32)] : per-ig chunk of 8 source slots
        inp("wsrc", [16, 128, 8 * K * 2 * 128], BF16)
        # wt[q, gg, (jl r)=128, (gl4 il4 d256)]
        inp("wtgt", [K, 8, 128, 4 * 4 * D], BF16)
    else:
        # ws[ig, dp, (il4 j16 dt2 r32)] : per-ig chunk of 4 source slots
        inp("wsrc", [32, 128, 4 * JL * 2 * R], BF16)
        # wt[q, gg, (il r)=128, (igl4 jl4 d256)]
        inp("wtgt", [K, 8, 128, 4 * 4 * D], BF16)
    inp("woutT", [NVT, 128, 2 * VT], BF16)
    io["lg_out"] = nc.dram_tensor(
        "lg_out", [L, V], BF16, kind="ExternalOutput"
    ).ap()
    io["dbg"] = nc.dram_tensor(
        "dbg", [128, 2048], F32, kind="ExternalOutput"
    ).ap()
    with tile.TileContext(nc) as tc:
        _body(nc, tc, io)
    nc.compile()
    return nc


def _body(nc, tc, io):
    with tc.tile_pool(name="const", bufs=1) as const, \
         tc.tile_pool(name="state", bufs=1) as state:

        ident = const.tile([128, 128], F32)
        masks.make_identity(nc, ident[:])
        ones = const.tile([128, 1], F32)
        nc.vector.memset(ones[:], 1.0)
        ones_row = const.tile([1, 128], F32)
        nc.vector.memset(ones_row[:], 1.0)
        eps_sb = const.tile([128, 1], F32)
        nc.vector.memset(eps_sb[:], LN_EPS)

        pid = nc.sync.partition_id()

        # persistent state
        hT16 = [state.tile([128, S, B], BF16, name=f"hT{dt}") for dt in range(2)]
        # own 16 slots, transposed: [dp, dt, jl, b]
        hown32 = state.tile([128, 2, JL, B], F32)
        hown16 = state.tile([128, 2, JL, B], BF16)
        qoT = [state.tile([128, L], F32, name=f"qoT{pt}") for pt in range(2)]
        lngT_sb = state.tile([128, STEPS, 2, 2], F32)
        nc.sync.dma_start(lngT_sb[:], io["lngT"])

        _phase_a(nc, tc, io, ident, ones, pid, hT16, hown32, hown16, qoT)
        nc.sync.dma_start(
            io["dbg"][:, 0:256],
            hown32[:].rearrange("p dt jl b -> p (dt jl b)"),
        )
        for t in range(N_STEPS):
            _step(nc, tc, t, io["wsrc"], io["wtgt"], hT16, hown32, hown16,
                  lngT_sb, ident, ones, ones_row, eps_sb,
                  io["dbg"] if t == 0 else None)
        _phase_c(nc, tc, io, ident, pid, hT16, qoT)


def _phase_a(nc, tc, io, ident, ones, pid, hT16, hown32, hown16, qoT):
    with tc.tile_pool(name="pa_sb", bufs=1) as pa, \
         tc.tile_pool(name="pa_ps", bufs=3, space="PSUM") as pps, \
         tc.tile_pool(name="pa_tp", bufs=2, space="PSUM") as tps, \
         tc.tile_pool(name="pa_acc", bufs=1, space="PSUM") as aps, \
         tc.tile_pool(name="dram_a", bufs=1, space="DRAM") as dra:

        mask_sb = pa.tile([128, 4], F32)
        nc.sync.dma_start(mask_sb[:], io["maskw"])

        # X^T tiles [d128, t512] (host-gathered embeddings, transposed)
        xT = [pa.tile([128, L], F32, name=f"xT{ct}") for ct in range(2)]
        for ct in range(2):
            nc.sync.dma_start(xT[ct][:], io["xT_in"][128 * ct : 128 * (ct + 1), :])

        # weight tiles [d128, 256] (contraction on partitions)
        def load_w(name):
            ts = [pa.tile([128, D], F32, name=f"{name}_{ct}") for ct in range(2)]
            for ct in range(2):
                nc.sync.dma_start(ts[ct][:], io[name][128 * ct : 128 * (ct + 1), :])
            return ts

        wq_sb = load_w("wqT")
        wv_sb = load_w("wvT")
        wksl_sb = load_w("wkslT")
        wqo_sb = load_w("wqoT")
        hTt = [pa.tile([128, S], F32, name=f"hTt{ct}") for ct in range(2)]
        for ct in range(2):
            nc.sync.dma_start(hTt[ct][:], io["hT_in"][128 * ct : 128 * (ct + 1), :])
        hn_sb = pa.tile([S, D], F32)
        nc.sync.dma_start(hn_sb[:], io["hn_in"])

        # Q_in^T and Q_out^T : [d'128 x 2, t512]
        qT = [pa.tile([128, L], F32, name=f"qT{pt}") for pt in range(2)]
        for pt in range(2):
            for dst, wsb in ((qT, wq_sb), (qoT, wqo_sb)):
                ps = pps.tile([128, L], F32, tag="ps")
                for ct in range(2):
                    nc.tensor.matmul(
                        ps[:], wsb[ct][:, 128 * pt : 128 * (pt + 1)], xT[ct][:],
                        start=(ct == 0), stop=(ct == 1),
                    )
                nc.vector.tensor_copy(dst[pt][:], ps[:])

        # V_in natural [t128 x 4, d256]
        vn = pa.tile([128, 4, D], F32)
        for tt in range(4):
            ps = pps.tile([128, L], F32, tag="ps")
            for ct in range(2):
                nc.tensor.matmul(
                    ps[:, 0:D], xT[ct][:, 128 * tt : 128 * (tt + 1)], wv_sb[ct][:],
                    start=(ct == 0), stop=(ct == 1),
                )
            nc.vector.tensor_copy(vn[:, tt, :], ps[:, 0:D])

        # K_slots^T [d'128 x 2, s128]
        kslT = [pa.tile([128, S], F32, name=f"kslT{pt}") for pt in range(2)]
        for pt in range(2):
            ps = pps.tile([128, L], F32, tag="ps")
            for ct in range(2):
                nc.tensor.matmul(
                    ps[:, 0:S], wksl_sb[ct][:, 128 * pt : 128 * (pt + 1)], hTt[ct][:],
                    start=(ct == 0), stop=(ct == 1),
                )
            nc.vector.tensor_copy(kslT[pt][:], ps[:, 0:S])

        # attention scores + masked softmax
        a_sb = pa.tile([128, 4, S], F32)
        for tt in range(4):
            sc = pps.tile([128, L], F32, tag="ps")
            for pt in range(2):
                nc.tensor.matmul(
                    sc[:, 0:S], qT[pt][:, 128 * tt : 128 * (tt + 1)], kslT[pt][:],
                    start=(pt == 0), stop=(pt == 1),
                )
            rowmax = pa.tile([128, 1], F32, tag="rmax")
            nc.vector.tensor_reduce(
                rowmax[:], sc[:, 0:S], axis=mybir.AxisListType.X,
                op=mybir.AluOpType.max,
            )
            nb = pa.tile([128, 1], F32, tag="nb")
            nc.vector.tensor_scalar_mul(nb[:], rowmax[:], -SCALE)
            sumexp = pa.tile([128, 1], F32, tag="sexp")
            nc.scalar.activation(
                a_sb[:, tt, :], sc[:, 0:S], mybir.ActivationFunctionType.Exp,
                bias=nb[:], scale=SCALE, accum_out=sumexp[:],
            )
            rs = pa.tile([128, 1], F32, tag="rs")
            nc.vector.reciprocal(rs[:], sumexp[:])
            rm = pa.tile([128, 1], F32, tag="rmk")
            nc.vector.tensor_tensor(
                rm[:], rs[:], mask_sb[:, tt : tt + 1], op=mybir.AluOpType.mult
            )
            nc.vector.tensor_scalar_mul(a_sb[:, tt, :], a_sb[:, tt, :], rm[:])

        # column sums and IR = A^T @ V
        cs = aps.tile([128, 1], F32, tag="cs")
        for tt in range(4):
            nc.tensor.matmul(
                cs[:], a_sb[:, tt, :], ones[:, 0:1], start=(tt == 0), stop=(tt == 3)
            )
        ir = aps.tile([128, D], F32, tag="ir")
        for tt in range(4):
            nc.tensor.matmul(
                ir[:], a_sb[:, tt, :], vn[:, tt, :], start=(tt == 0), stop=(tt == 3)
            )
        cssb = pa.tile([128, 1], F32)
        nc.vector.tensor_scalar_add(cssb[:], cs[:], 1e-8)
        rcs = pa.tile([128, 1], F32)
        nc.vector.reciprocal(rcs[:], cssb[:])
        h0 = pa.tile([S, D], F32)
        nc.vector.scalar_tensor_tensor(
            h0[:], ir[:], rcs[:], hn_sb[:],
            op0=mybir.AluOpType.mult, op1=mybir.AluOpType.add,
        )

        # h0 -> transposed bounce (bf16), AllGather
        agin0 = dra.tile([2 * 16384], BF16)
        for dt in range(2):
            p3 = tps.tile([128, 128], F32, tag="tp")
            nc.tensor.transpose(p3[:], h0[:, 128 * dt : 128 * (dt + 1)], ident[:])
            h0T = pa.tile([128, 128], BF16, tag="h0T")
            nc.vector.tensor_copy(h0T[:], p3[:])
            nc.sync.dma_start(
                agin0[dt * 16384 : (dt + 1) * 16384].rearrange(
                    "(p f) -> p f", p=128
                ),
                h0T[:],
            )
        agout0 = dra.tile([NC, 2 * 16384], BF16, addr_space="Shared")
        nc.gpsimd.collective_compute(
            "AllGather", mybir.AluOpType.bypass,
            ins=[agin0[:].opt()], outs=[agout0[:].opt()],
            replica_groups=[list(range(NC))],
        )
        # readback: hT16[dt][dp, s, b] ; hown16/32 (own transposed slice via pid)
        ag0r = agout0[:].rearrange(
            "b (seg dp s) -> seg dp s b", seg=2, dp=128, s=128
        )
        pid_v = nc.vector.partition_id()
        for dt in range(2):
            nc.sync.dma_start(hT16[dt][:], ag0r[dt])
            nc.vector.tensor_copy(
                hown16[:, dt, :, :].rearrange("p jl b -> p (jl b)"),
                hT16[dt][:].rearrange("p s b -> p (s b)")[
                    :, bass.ds(pid_v * (JL * B), JL * B)
                ],
            )
        nc.vector.tensor_copy(hown32[:], hown16[:])


def _step(nc, tc, t, wsrc, wtgt, hT16, hown32, hown16, lngT_sb, ident, ones,
          ones_row, eps_sb, dbg=None):
    """One message-passing step."""
    with tc.tile_pool(name=f"s{t}_ws", bufs=8) as wsp, \
         tc.tile_pool(name=f"s{t}_wt", bufs=4) as wtp, \
         tc.tile_pool(name=f"s{t}_sb", bufs=1) as sb, \
         tc.tile_pool(name=f"s{t}_p1", bufs=1, space="PSUM") as p1p, \
         tc.tile_pool(name=f"s{t}_p2", bufs=1, space="PSUM") as p2p, \
         tc.tile_pool(name=f"s{t}_p3", bufs=1, space="PSUM") as p3p, \
         tc.tile_pool(name=f"s{t}_p4", bufs=1, space="PSUM") as p4p, \
         tc.tile_pool(name=f"s{t}_dram", bufs=1, space="DRAM") as drp:

        if LAYOUT == "Q":
            inter = sb.tile([128, K, S, B], BF16)
            # ---- einsum1: inter[(jl,r), q, i, b] = h[b,i,:] @ Ws[i, j] ----
            for ig in range(16):
                ws = wsp.tile([128, 8, K, 2, 128], BF16, tag="ws")
                nc.sync.dma_start(ws[:], wsrc[ig].rearrange("dp (i q dt jr) -> dp i q dt jr", i=8, q=K, dt=2))
                for i8 in range(8):
                    i = 8 * ig + i8
                    p1 = p1p.tile([128, K, B], F32, tag="p1")
                    for q in range(K):
                        for dt in range(2):
                            nc.tensor.matmul(
                                p1[:, q, :],
                                ws[:, i8, q, dt, :],
                                hT16[dt][:, i, :],
                                start=(dt == 0), stop=(dt == 1),
                            )
                    nc.vector.tensor_copy(inter[:, :, i, :], p1[:])
        else:
            # ---- einsum1: inter2[(il,r), ig, j, b] via column tile_position ----
            inter2 = sb.tile([128, 32, JL, B], BF16)
            for ig in range(32):
                ws = wsp.tile([128, 4, JL, 2, R], BF16, tag="ws")
                nc.sync.dma_start(ws[:], wsrc[ig].rearrange("dp (il j dt r) -> dp il j dt r", il=4, j=JL, dt=2))
                p1 = p1p.tile([128, JL, B], F32, tag="p1")
                for il in range(4):
                    i = 4 * ig + il
                    for j in range(JL):
                        for dt in range(2):
                            nc.tensor.matmul(
                                p1[32 * il : 32 * (il + 1), j, :],
                                ws[:, il, j, dt, :],
                                hT16[dt][:, i, :],
                                start=(dt == 0), stop=(dt == 1),
                                tile_position=(0, 32 * il),
                            )
                nc.vector.tensor_copy(inter2[:, ig, :, :], p1[:])

        # ---- einsum2 per quad q; relu + transpose into tcandT ----
        tcandT = sb.tile([128, 2, JL, B], F32)   # relu(infl)^T
        for q in range(K):
            # one PSUM bank per jl — concurrent row tiles must not share banks
            p2 = [p2p.tile([B, D], F32, tag=f"p2_{jl}", name=f"p2_{jl}")[:] for jl in range(4)]
            for gg in range(8):
                wt = wtp.tile([128, 4, 4, D], BF16, tag="wt")
                nc.sync.dma_start(wt[:], wtgt[q, gg].rearrange("p (gl il d) -> p gl il d", gl=4, il=4))
                if LAYOUT == "Q":
                    for gl in range(4):
                        for il in range(4):
                            i = 16 * gg + 4 * gl + il
                            for jl in range(4):
                                nc.tensor.matmul(
                                    p2[jl],
                                    inter[32 * jl : 32 * (jl + 1), q, i, :],
                                    wt[32 * jl : 32 * (jl + 1), gl, il, :],
                                    start=(i == 0), stop=(i == S - 1),
                                    tile_position=(32 * jl, 0),
                                )
                else:
                    for igl in range(4):
                        ig = 4 * gg + igl
                        for jl in range(4):
                            nc.tensor.matmul(
                                p2[jl],
                                inter2[:, ig, 4 * q + jl, :],
                                wt[:, igl, jl, :],
                                start=(ig == 0), stop=(ig == 31),
                            )
            hrelu = sb.tile([B, 4, D], F32, tag="hrelu")
            for jl in range(4):
                nc.scalar.activation(
                    hrelu[:, jl, :], p2[jl],
                    mybir.ActivationFunctionType.Relu,
                )
            for jloc in range(4):
                for dt in range(2):
                    p3 = p3p.tile([128, B], F32, tag="p3")
                    nc.tensor.transpose(
                        p3[:],
                        hrelu[:, jloc, 128 * dt : 128 * (dt + 1)],
                        ident[0:B, 0:B],
                    )
                    nc.vector.tensor_copy(
                        tcandT[:, dt, 4 * q + jloc, :], p3[:]
                    )

        if dbg is not None:
            nc.sync.dma_start(
                dbg[:, 256:512],
                tcandT[:].rearrange("p dt jl b -> p (dt jl b)"),
            )
        # ---- residual + LayerNorm in transposed layout (128 lanes) ----
        # hx holds x and x^2 side by side so ONE accumulation group computes
        # both partition sums (interleaved psum groups sharing a bank lose
        # the earlier group's partial on the later group's start).
        hx = sb.tile([128, 2, 2, JL, B], F32, tag="hx")  # [dp, dt, {x,x2}, jl, b]
        hcand = hx[:, :, 0, :, :]
        nc.vector.tensor_tensor(
            hcand, tcandT[:], hown32[:], op=mybir.AluOpType.add
        )
        if dbg is not None:
            nc.sync.dma_start(
                dbg[:, 512:768].rearrange("p (dt f) -> p dt f", dt=2),
                hcand.rearrange("p dt jl b -> p dt (jl b)"),
            )
        nc.vector.tensor_tensor(
            hx[:, :, 1, :, :], hcand, hcand, op=mybir.AluOpType.mult
        )
        s12 = p4p.tile([1, 2, JL * B], F32, tag="s12")
        for dt in range(2):
            nc.tensor.matmul(
                s12[:].rearrange("o c f -> o (c f)"), ones[:, 0:1],
                hx[:, dt, :, :, :].rearrange("p c jl b -> p (c jl b)"),
                start=(dt == 0), stop=(dt == 1),
            )
        mean = sb.tile([1, JL * B], F32, tag="mean")
        nc.vector.tensor_scalar_mul(mean[:], s12[:, 0, :], 1.0 / D)
        ex2 = sb.tile([1, JL * B], F32, tag="ex2")
        nc.vector.tensor_scalar_mul(ex2[:], s12[:, 1, :], 1.0 / D)
        m2 = sb.tile([1, JL * B], F32, tag="m2")
        nc.vector.tensor_tensor(m2[:], mean[:], mean[:], op=mybir.AluOpType.mult)
        var = sb.tile([1, JL * B], F32, tag="var")
        nc.vector.tensor_tensor(var[:], ex2[:], m2[:], op=mybir.AluOpType.subtract)
        std = sb.tile([1, JL * B], F32, tag="std")
        nc.scalar.activation(
            std[:], var[:], mybir.ActivationFunctionType.Sqrt,
            bias=eps_sb[0:1, :], scale=1.0,
        )
        # mean/rstd concat -> single broadcast matmul (one group)
        mr = sb.tile([1, 2, JL * B], F32, tag="mr")
        nc.vector.tensor_copy(mr[:, 0, :], mean[:])
        nc.vector.reciprocal(mr[:, 1, :], std[:])
        rstd = mr[:, 1, :]
        mbrb = p3p.tile([128, 2, JL * B], F32, tag="mbrb")
        nc.tensor.matmul(
            mbrb[:].rearrange("p c f -> p (c f)"), ones_row[:],
            mr[:].rearrange("o c f -> o (c f)"),
            start=True, stop=True,
        )
        t1 = sb.tile([128, 2, JL, B], F32, tag="t1")
        mb_r = mbrb[:, 0, :].rearrange("p (jl b) -> p jl b", jl=JL)
        rb_r = mbrb[:, 1, :].rearrange("p (jl b) -> p jl b", jl=JL)
        for dt in range(2):
            nc.vector.tensor_tensor(
                t1[:, dt, :, :], hcand[:, dt, :, :], mb_r,
                op=mybir.AluOpType.subtract,
            )
            nc.vector.tensor_tensor(
                t1[:, dt, :, :], t1[:, dt, :, :], rb_r,
                op=mybir.AluOpType.mult,
            )
            nc.vector.tensor_scalar(
                hown32[:, dt, :, :], t1[:, dt, :, :],
                lngT_sb[:, t, dt, 0:1], lngT_sb[:, t, dt, 1:2],
                op0=mybir.AluOpType.mult, op1=mybir.AluOpType.add,
            )
        nc.vector.tensor_copy(hown16[:], hown32[:])
        if dbg is not None:
            nc.sync.dma_start(
                dbg[:, 1024:1280],
                hown32[:].rearrange("p dt jl b -> p (dt jl b)"),
            )
            nc.sync.dma_start(dbg[0:1, 768:896], mean[:])
            nc.sync.dma_start(dbg[0:1, 896:1024], rstd)

        # ---- AllGather the own transposed slots (bf16); rebuild hT16 ----
        agin = drp.tile([2 * 128 * JL * B], BF16)
        for dt in range(2):
            nc.sync.dma_start(
                agin[dt * 16384 : (dt + 1) * 16384].rearrange(
                    "(p f) -> p f", p=128
                ),
                hown16[:, dt, :, :],
            )
        agout = drp.tile([NC, 2 * 128 * JL * B], BF16, addr_space="Shared")
        nc.gpsimd.collective_compute(
            "AllGather", mybir.AluOpType.bypass,
            ins=[agin[:].opt()], outs=[agout[:].opt()],
            replica_groups=[list(range(NC))],
        )
        agr = agout[:].rearrange(
            "rk (dt dp jl b) -> dt dp rk jl b", dt=2, dp=128, jl=JL, b=B
        )
        for dt in range(2):
            nc.sync.dma_start(
                hT16[dt][:].rearrange("dp (rk jl) b -> dp rk jl b", rk=NC), agr[dt]
            )


def _phase_c(nc, tc, io, ident, pid, hT16, qoT):
    with tc.tile_pool(name="pc_sb", bufs=1) as pc, \
         tc.tile_pool(name="pc_ps", bufs=3, space="PSUM") as cps, \
         tc.tile_pool(name="pc_lg", bufs=4, space="PSUM") as lgps, \
         tc.tile_pool(name="pc_wo", bufs=4) as wop:

        wkf_sb = [pc.tile([128, D], F32, name=f"wkf{ct}") for ct in range(2)]
        wvf_sb = [pc.tile([128, D], F32, name=f"wvf{ct}") for ct in range(2)]
        for ct in range(2):
            nc.sync.dma_start(
                wkf_sb[ct][:], io["wkfT"][128 * ct : 128 * (ct + 1), :]
            )
            nc.sync.dma_start(
                wvf_sb[ct][:], io["wvfT"][128 * ct : 128 * (ct + 1), :]
            )

        # own-batch h^T slice (dynamic b=pid) -> static f32 tiles
        pid_v = nc.vector.partition_id()
        hb = [pc.tile([128, S], F32, name=f"hb{dt}") for dt in range(2)]
        for dt in range(2):
            nc.vector.tensor_copy(
                hb[dt][:].rearrange("p (s o) -> p s o", o=1),
                hT16[dt][:, :, bass.ds(pid_v, 1)],
            )

        # K_f^T [d'128 x2, s128] ; V_f natural [s, d']
        kfT = [pc.tile([128, S], F32, name=f"kfT{pt}") for pt in range(2)]
        for pt in range(2):
            ps = cps.tile([128, L], F32, tag="c")
            for ct in range(2):
                nc.tensor.matmul(
                    ps[:, 0:S], wkf_sb[ct][:, 128 * pt : 128 * (pt + 1)], hb[ct][:],
                    start=(ct == 0), stop=(ct == 1),
                )
            nc.vector.tensor_copy(kfT[pt][:], ps[:, 0:S])
        vf = pc.tile([S, D], F32)
        psv = cps.tile([128, L], F32, tag="c")
        for ct in range(2):
            nc.tensor.matmul(
                psv[0:S, 0:D], hb[ct][:], wvf_sb[ct][:],
                start=(ct == 0), stop=(ct == 1),
            )
        nc.vector.tensor_copy(vf[:], psv[0:S, 0:D])

        # expand attention -> A2^T [s, t512]
        a2T = pc.tile([S, L], F32)
        for tt in range(4):
            sc = cps.tile([128, L], F32, tag="c")
            for pt in range(2):
                nc.tensor.matmul(
                    sc[:, 0:S], qoT[pt][:, 128 * tt : 128 * (tt + 1)], kfT[pt][:],
                    start=(pt == 0), stop=(pt == 1),
                )
            rowmax = pc.tile([128, 1], F32, tag="rmax2")
            nc.vector.tensor_reduce(
                rowmax[:], sc[:, 0:S], axis=mybir.AxisListType.X,
                op=mybir.AluOpType.max,
            )
            nb = pc.tile([128, 1], F32, tag="nb2")
            nc.vector.tensor_scalar_mul(nb[:], rowmax[:], -SCALE)
            a2 = pc.tile([128, S], F32, tag="a2")
            sumexp = pc.tile([128, 1], F32, tag="sexp2")
            nc.scalar.activation(
                a2[:], sc[:, 0:S], mybir.ActivationFunctionType.Exp,
                bias=nb[:], scale=SCALE, accum_out=sumexp[:],
            )
            rs = pc.tile([128, 1], F32, tag="rs2")
            nc.vector.reciprocal(rs[:], sumexp[:])
            nc.vector.tensor_scalar_mul(a2[:], a2[:], rs[:])
            ptr = cps.tile([128, L], F32, tag="c")
            nc.tensor.transpose(ptr[:, 0:S], a2[:], ident[:])
            nc.vector.tensor_copy(a2T[:, 128 * tt : 128 * (tt + 1)], ptr[:, 0:S])

        # Y^T [d128 x2, t512] -> bf16
        yT16 = [pc.tile([128, L], BF16, name=f"yT{dt}") for dt in range(2)]
        for dt in range(2):
            ps = cps.tile([128, L], F32, tag="c")
            nc.tensor.matmul(
                ps[:], vf[:, 128 * dt : 128 * (dt + 1)], a2T[:],
                start=True, stop=True,
            )
            nc.vector.tensor_copy(yT16[dt][:], ps[:])

        # logits tiles (bf16 matmul) + one coalesced bf16 store per vocab tile
        lg_dst = io["lg_out"].rearrange("(tt p) v -> p tt v", tt=4)
        for vt in range(NVT):
            wo_sb = wop.tile([128, 2, VT], BF16, tag="wo")
            nc.sync.dma_start(
                wo_sb[:], io["woutT"][vt].rearrange("dp (dt v) -> dp dt v", dt=2)
            )
            lg_sb = wop.tile([128, 4, VT], BF16, tag="lg_sb", name="lg_sb")
            for tt in range(4):
                lg = lgps.tile([128, VT], F32, tag="lg")
                for dt in range(2):
                    nc.tensor.matmul(
                        lg[:],
                        yT16[dt][:, 128 * tt : 128 * (tt + 1)],
                        wo_sb[:, dt, :],
                        start=(dt == 0), stop=(dt == 1),
                    )
                nc.any.tensor_copy(lg_sb[:, tt, :], lg[:])
            nc.sync.dma_start(
                lg_dst[:, :, VT * vt : VT * (vt + 1)], lg_sb[:]
            )


# ---------------------------------------------------------------------------
# Host side
# ---------------------------------------------------------------------------

_NC_CACHE = {}


def _get_nc():
    key = (N_STEPS, LAYOUT)
    if key not in _NC_CACHE:
        _NC_CACHE[key] = build()
    return _NC_CACHE[key]


def _prep_in_maps(inputs):
    f32 = lambda a: np.ascontiguousarray(np.asarray(a), dtype=np.float32)
    input_ids = np.asarray(inputs["input_ids"])
    attention_mask = np.asarray(inputs["attention_mask"])
    H = f32(inputs["H"])
    W_source = f32(inputs["W_source"])
    W_target = f32(inputs["W_target"])

    # lngT[dp, t, dt, {scale, bias}] = ln_{scale,bias}[t, 128dt+dp]
    lngT = np.stack(
        [
            f32(inputs["ln_scale"]).reshape(STEPS, 2, 128).transpose(2, 0, 1),
            f32(inputs["ln_bias"]).reshape(STEPS, 2, 128).transpose(2, 0, 1),
        ],
        axis=-1,
    )

    rep = {
        "wqT": f32(np.asarray(inputs["Wq_in"]).T),
        "wkslT": f32(np.asarray(inputs["Wk_slots"]).T),
        "wvT": f32(np.asarray(inputs["Wv_in"]).T),
        "wqoT": f32(np.asarray(inputs["Wq_out"]).T),
        "wkfT": f32(np.asarray(inputs["Wk_fin"]).T),
        "wvfT": f32(np.asarray(inputs["Wv_fin"]).T),
        "hT_in": f32(H.T),
        "hn_in": H,
        "lngT": np.ascontiguousarray(lngT),
        # woutT[vt, dp, (dt, vl)] = Wout[500vt+vl, 128dt+dp]  (bf16)
        "woutT": np.ascontiguousarray(
            f32(inputs["W_out_proj"]).reshape(NVT, VT, 2, 128)
            .transpose(0, 3, 2, 1).reshape(NVT, 128, 2 * VT)
        ).astype(NP_BF16),
    }

    in_maps = []
    for c in range(NC):
        m = dict(rep)
        X = (np.asarray(inputs["token_emb"], dtype=np.float32)[input_ids[c]]
             + np.asarray(inputs["pos_emb"], dtype=np.float32))
        m["xT_in"] = np.ascontiguousarray(X.T)
        m["maskw"] = np.ascontiguousarray(
            attention_mask[c].astype(np.float32).reshape(4, 128).T
        )
        ws = W_source[:, JL * c : JL * (c + 1)]      # [S, 16, D, R]
        # The reference masks out the i == j (diagonal) pair; zeroing
        # W_target[j, j] is exactly equivalent since the term is linear in it.
        wt = W_target[:, JL * c : JL * (c + 1)].copy()   # [S, 16, R, D]
        for jl in range(JL):
            wt[JL * c + jl, jl] = 0.0
        if LAYOUT == "Q":
            # ws[ig, dp, (i8, q, dt, jl, r)] = Ws[8ig+i8, 16c+4q+jl, 128dt+dp, r]
            ws = ws.reshape(16, 8, K, 4, 2, 128, R).transpose(0, 5, 1, 2, 4, 3, 6)
            m["wsrc"] = np.ascontiguousarray(ws).reshape(
                16, 128, 8 * K * 2 * 128
            ).astype(NP_BF16)
            # wt[q, gg, (jl, r), (gl, il, d)] = Wt[16gg+4gl+il, 16c+4q+jl, r, d]
            wt = wt.reshape(8, 4, 4, K, 4, R, D).transpose(3, 0, 4, 5, 1, 2, 6)
            m["wtgt"] = np.ascontiguousarray(wt).reshape(
                K, 8, 128, 4 * 4 * D
            ).astype(NP_BF16)
        else:
            # ws[ig, dp, (il, j, dt, r)] = Ws[4ig+il, 16c+j, 128dt+dp, r]
            ws = ws.reshape(32, 4, JL, 2, 128, R).transpose(0, 4, 1, 2, 3, 5)
            m["wsrc"] = np.ascontiguousarray(ws).reshape(
                32, 128, 4 * JL * 2 * R
            ).astype(NP_BF16)
            # wt[q, gg, (il, r), (igl, jl, d)] = Wt[16gg+4igl+il, 16c+4q+jl, r, d]
            wt = wt.reshape(8, 4, 4, K, 4, R, D).transpose(3, 0, 2, 5, 1, 4, 6)
            m["wtgt"] = np.ascontiguousarray(wt).reshape(
                K, 8, 128, 4 * 4 * D
            ).astype(NP_BF16)
        in_maps.append(m)
    return in_maps


def run(inputs, trace=False):
    nc = _get_nc()
    in_maps = _prep_in_maps(inputs)
    res = bass_utils.run_bass_kernel_spmd(
        nc, in_maps, core_ids=list(range(NC)), trace=trace
    )
    out = np.stack(
        [res.results[c]["lg_out"].astype(np.float32) for c in range(NC)], axis=0
    )
    return out, res


def kernel(**inputs):
    out, _ = run(inputs, trace=False)
    return out


# revision 48
# speedup vs baseline: 39.4741x; 1.0373x over previous
"""Trainium2 Bass kernel for nn_ConnectionTransformer (8 NeuronCores, SPMD).

Strategy
--------
- Phase A (embed + compress attention): batch-parallel, core c handles batch c.
- Phase B (6 bilinear message-passing steps): target-slot sharding — core c owns
  16 target slots j in [16c, 16c+16). Each core computes the full influence for
  its slots (sum over all source slots i), applies relu/residual/LayerNorm
  locally, and an AllGather rebuilds the replicated transposed state h^T each
  step. The per-pair weights W_source/W_target are cast to bf16 on the host
  (512 MB total), sharded along j and streamed from HBM once per step per core
  (32+32 MB) in large contiguous DMAs — the memory roofline.
- Phase C (expand attention + vocab projection): batch-parallel again; W_out in
  bf16, logits emitted as bf16 and upcast on the host.

All weights are pre-transposed/tiled on the host into matmul-ready layouts so
the device never transposes weight tensors. All big matmuls run in bf16
(full-rate PE, half DMA traffic); LayerNorm/softmax accumulate in fp32.
"""
import os
import sys

sys.path.insert(0, "/opt/trn_rl_repo")

import numpy as np
import ml_dtypes
from concourse import bass, bacc, tile, bass_utils, mybir
from concourse import masks

B, L, D, S, R, STEPS, V = 8, 512, 256, 128, 32, 6, 32000
NC = 8
JL = S // NC          # 16 local target slots per core
K = JL // 4           # 4 quads of target slots
VT = 500              # vocab tile width
NVT = V // VT         # 64 vocab tiles
SCALE = 1.0 / np.sqrt(D)
LN_EPS = 1e-5

F32 = mybir.dt.float32
BF16 = mybir.dt.bfloat16
NP_BF16 = ml_dtypes.bfloat16

N_STEPS = int(os.environ.get("N_STEPS", str(STEPS)))
# Q: einsum2 contracts 32 (r) with 4-way row tile_position packing.
# P: einsum1 scatters to (il,r) partitions via column tile_position;
#    einsum2 contracts 128 in 4x fewer matmuls. NOTE: broken on HW — the
#    il=3 column tile needs PE column quadrant 3, which cannot take weight
#    loads (known HW limitation), so results are silently wrong. Kept for
#    cost-model experiments only.
LAYOUT = os.environ.get("LAYOUT", "Q")


# ---------------------------------------------------------------------------
# Device program
# ---------------------------------------------------------------------------

def build():
    nc = bacc.Bacc("TRN2", target_bir_lowering=False, debug=False, num_devices=NC)

    io = {}

    def inp(name, shape, dtype=F32):
        io[name] = nc.dram_tensor(name, shape, dtype, kind="ExternalInput").ap()

    inp("xT_in", [D, L])
    inp("maskw", [128, 4])
    for w in ("wqT", "wkslT", "wvT", "wqoT", "wkfT", "wvfT"):
        inp(w, [D, D])
    inp("hT_in", [D, S])
    inp("hn_in", [S, D])
    inp("lngT", [128, STEPS, 2, 2])
    if LAYOUT == "Q":
        # ws[ig, dp, (i8 q4 dt2 jl4 r32)] : per-ig chunk of 8 source slots
        inp("wsrc", [16, 128, 8 * K * 2 * 128], BF16)
        # wt[q, gg, (jl r)=128, (gl4 il4 d256)]
        inp("wtgt", [K, 8, 128, 4 * 4 * D], BF16)
    else:
        # ws[ig, dp, (il4 j16 dt2 r32)] : per-ig chunk of 4 source slots
        inp("wsrc", [32, 128, 4 * JL * 2 * R], BF16)
        # wt[q, gg, (il r)=128, (igl4 jl4 d256)]
        inp("wtgt", [K, 8, 128, 4 * 4 * D], BF16)
    inp("woutT", [NVT, 128, 2 * VT], BF16)
    io["lg_out"] = nc.dram_tensor(
        "lg_out", [L, V], BF16, kind="ExternalOutput"
    ).ap()
    io["dbg"] = nc.dram_tensor(
        "dbg", [128, 2048], F32, kind="ExternalOutput"
    ).ap()
    with tile.TileContext(nc) as tc:
        _body(nc, tc, io)
    nc.compile()
    return nc


def _body(nc, tc, io):
    with tc.tile_pool(name="const", bufs=1) as const, \
         tc.tile_pool(name="state", bufs=1) as state:

        ident = const.tile([128, 128], F32)
        masks.make_identity(nc, ident[:])
        ones = const.tile([128, 1], F32)
        nc.vector.memset(ones[:], 1.0)
        ones_row = const.tile([1, 128], F32)
        nc.vector.memset(ones_row[:], 1.0)
        eps_sb = const.tile([128, 1], F32)
        nc.vector.memset(eps_sb[:], LN_EPS)

        pid = nc.sync.partition_id()

        # persistent state
        hT16 = [state.tile([128, S, B], BF16, name=f"hT{dt}") for dt in range(2)]
        # own 16 slots, transposed: [dp, dt, jl, b]
        hown32 = state.tile([128, 2, JL, B], F32)
        hown16 = state.tile([128, 2, JL, B], BF16)
        qoT = [state.tile([128, L], F32, name=f"qoT{pt}") for pt in range(2)]
        lngT_sb = state.tile([128, STEPS, 2, 2], F32)
        nc.sync.dma_start(lngT_sb[:], io["lngT"])

        _phase_a(nc, tc, io, ident, ones, pid, hT16, hown32, hown16, qoT)
        nc.sync.dma_start(
            io["dbg"][:, 0:256],
            hown32[:].rearrange("p dt jl b -> p (dt jl b)"),
        )
        for t in range(N_STEPS):
            _step(nc, tc, t, io["wsrc"], io["wtgt"], hT16, hown32, hown16,
                  lngT_sb, ident, ones, ones_row, eps_sb,
                  io["dbg"] if t == 0 else None)
        _phase_c(nc, tc, io, ident, pid, hT16, qoT)


def _phase_a(nc, tc, io, ident, ones, pid, hT16, hown32, hown16, qoT):
    with tc.tile_pool(name="pa_sb", bufs=1) as pa, \
         tc.tile_pool(name="pa_ps", bufs=3, space="PSUM") as pps, \
         tc.tile_pool(name="pa_tp", bufs=2, space="PSUM") as tps, \
         tc.tile_pool(name="pa_acc", bufs=1, space="PSUM") as aps, \
         tc.tile_pool(name="dram_a", bufs=1, space="DRAM") as dra:

        mask_sb = pa.tile([128, 4], F32)
        nc.sync.dma_start(mask_sb[:], io["maskw"])

        # X^T tiles [d128, t512] (host-gathered embeddings, transposed)
        xT = [pa.tile([128, L], F32, name=f"xT{ct}") for ct in range(2)]
        for ct in range(2):
            nc.sync.dma_start(xT[ct][:], io["xT_in"][128 * ct : 128 * (ct + 1), :])

        # weight tiles [d128, 256] (contraction on partitions)
        def load_w(name):
            ts = [pa.tile([128, D], F32, name=f"{name}_{ct}") for ct in range(2)]
            for ct in range(2):
                nc.sync.dma_start(ts[ct][:], io[name][128 * ct : 128 * (ct + 1), :])
            return ts

        wq_sb = load_w("wqT")
        wv_sb = load_w("wvT")
        wksl_sb = load_w("wkslT")
        wqo_sb = load_w("wqoT")
        hTt = [pa.tile([128, S], F32, name=f"hTt{ct}") for ct in range(2)]
        for ct in range(2):
            nc.sync.dma_start(hTt[ct][:], io["hT_in"][128 * ct : 128 * (ct + 1), :])
        hn_sb = pa.tile([S, D], F32)
        nc.sync.dma_start(hn_sb[:], io["hn_in"])

        # Q_in^T and Q_out^T : [d'128 x 2, t512]
        qT = [pa.tile([128, L], F32, name=f"qT{pt}") for pt in range(2)]
        for pt in range(2):
            for dst, wsb in ((qT, wq_sb), (qoT, wqo_sb)):
                ps = pps.tile([128, L], F32, tag="ps")
                for ct in range(2):
                    nc.tensor.matmul(
                        ps[:], wsb[ct][:, 128 * pt : 128 * (pt + 1)], xT[ct][:],
                        start=(ct == 0), stop=(ct == 1),
                    )
                nc.vector.tensor_copy(dst[pt][:], ps[:])

        # V_in natural [t128 x 4, d256]
        vn = pa.tile([128, 4, D], F32)
        for tt in range(4):
            ps = pps.tile([128, L], F32, tag="ps")
            for ct in range(2):
                nc.tensor.matmul(
                    ps[:, 0:D], xT[ct][:, 128 * tt : 128 * (tt + 1)], wv_sb[ct][:],
                    start=(ct == 0), stop=(ct == 1),
                )
            nc.vector.tensor_copy(vn[:, tt, :], ps[:, 0:D])

        # K_slots^T [d'128 x 2, s128]
        kslT = [pa.tile([128, S], F32, name=f"kslT{pt}") for pt in range(2)]
        for pt in range(2):
            ps = pps.tile([128, L], F32, tag="ps")
            for ct in range(2):
                nc.tensor.matmul(
                    ps[:, 0:S], wksl_sb[ct][:, 128 * pt : 128 * (pt + 1)], hTt[ct][:],
                    start=(ct == 0), stop=(ct == 1),
                )
            nc.vector.tensor_copy(kslT[pt][:], ps[:, 0:S])

        # attention scores + masked softmax
        a_sb = pa.tile([128, 4, S], F32)
        for tt in range(4):
            sc = pps.tile([128, L], F32, tag="ps")
            for pt in range(2):
                nc.tensor.matmul(
                    sc[:, 0:S], qT[pt][:, 128 * tt : 128 * (tt + 1)], kslT[pt][:],
                    start=(pt == 0), stop=(pt == 1),
                )
            rowmax = pa.tile([128, 1], F32, tag="rmax")
            nc.vector.tensor_reduce(
                rowmax[:], sc[:, 0:S], axis=mybir.AxisListType.X,
                op=mybir.AluOpType.max,
            )
            nb = pa.tile([128, 1], F32, tag="nb")
            nc.vector.tensor_scalar_mul(nb[:], rowmax[:], -SCALE)
            sumexp = pa.tile([128, 1], F32, tag="sexp")
            nc.scalar.activation(
                a_sb[:, tt, :], sc[:, 0:S], mybir.ActivationFunctionType.Exp,
                bias=nb[:], scale=SCALE, accum_out=sumexp[:],
            )
            rs = pa.tile([128, 1], F32, tag="rs")
            nc.vector.reciprocal(rs[:], sumexp[:])
            rm = pa.tile([128, 1], F32, tag="rmk")
            nc.vector.tensor_tensor(
                rm[:], rs[:], mask_sb[:, tt : tt + 1], op=mybir.AluOpType.mult
            )
            nc.vector.tensor_scalar_mul(a_sb[:, tt, :], a_sb[:, tt, :], rm[:])

        # column sums and IR = A^T @ V
        cs = aps.tile([128, 1], F32, tag="cs")
        for tt in range(4):
            nc.tensor.matmul(
                cs[:], a_sb[:, tt, :], ones[:, 0:1], start=(tt == 0), stop=(tt == 3)
            )
        ir = aps.tile([128, D], F32, tag="ir")
        for tt in range(4):
            nc.tensor.matmul(
                ir[:], a_sb[:, tt, :], vn[:, tt, :], start=(tt == 0), stop=(tt == 3)
            )
        cssb = pa.tile([128, 1], F32)
        nc.vector.tensor_scalar_add(cssb[:], cs[:], 1e-8)
        rcs = pa.tile([128, 1], F32)
        nc.vector.reciprocal(rcs[:], cssb[:])
        h0 = pa.tile([S, D], F32)
        nc.vector.scalar_tensor_tensor(
            h0[:], ir[:], rcs[:], hn_sb[:],
            op0=mybir.AluOpType.mult, op1=mybir.AluOpType.add,
        )

        # h0 -> transposed bounce (bf16), AllGather
        agin0 = dra.tile([2 * 16384], BF16)
        for dt in range(2):
            p3 = tps.tile([128, 128], F32, tag="tp")
            nc.tensor.transpose(p3[:], h0[:, 128 * dt : 128 * (dt + 1)], ident[:])
            h0T = pa.tile([128, 128], BF16, tag="h0T")
            nc.vector.tensor_copy(h0T[:], p3[:])
            nc.sync.dma_start(
                agin0[dt * 16384 : (dt + 1) * 16384].rearrange(
                    "(p f) -> p f", p=128
                ),
                h0T[:],
            )
        agout0 = dra.tile([NC, 2 * 16384], BF16, addr_space="Shared")
        nc.gpsimd.collective_compute(
            "AllGather", mybir.AluOpType.bypass,
            ins=[agin0[:].opt()], outs=[agout0[:].opt()],
            replica_groups=[list(range(NC))],
        )
        # readback: hT16[dt][dp, s, b] ; hown16/32 (own transposed slice via pid)
        ag0r = agout0[:].rearrange(
            "b (seg dp s) -> seg dp s b", seg=2, dp=128, s=128
        )
        pid_v = nc.vector.partition_id()
        for dt in range(2):
            nc.sync.dma_start(hT16[dt][:], ag0r[dt])
            nc.vector.tensor_copy(
                hown16[:, dt, :, :].rearrange("p jl b -> p (jl b)"),
                hT16[dt][:].rearrange("p s b -> p (s b)")[
                    :, bass.ds(pid_v * (JL * B), JL * B)
                ],
            )
        nc.vector.tensor_copy(hown32[:], hown16[:])


def _step(nc, tc, t, wsrc, wtgt, hT16, hown32, hown16, lngT_sb, ident, ones,
          ones_row, eps_sb, dbg=None):
    """One message-passing step."""
    with tc.tile_pool(name=f"s{t}_ws", bufs=8) as wsp, \
         tc.tile_pool(name=f"s{t}_wt", bufs=5) as wtp, \
         tc.tile_pool(name=f"s{t}_sb", bufs=1) as sb, \
         tc.tile_pool(name=f"s{t}_p1", bufs=1, space="PSUM") as p1p, \
         tc.tile_pool(name=f"s{t}_p2", bufs=1, space="PSUM") as p2p, \
         tc.tile_pool(name=f"s{t}_p3", bufs=1, space="PSUM") as p3p, \
         tc.tile_pool(name=f"s{t}_p4", bufs=1, space="PSUM") as p4p, \
         tc.tile_pool(name=f"s{t}_dram", bufs=1, space="DRAM") as drp:

        if LAYOUT == "Q":
            inter = sb.tile([128, K, S, B], BF16)
            # ---- einsum1: inter[(jl,r), q, i, b] = h[b,i,:] @ Ws[i, j] ----
            for ig in range(16):
                ws = wsp.tile([128, 8, K, 2, 128], BF16, tag="ws")
                nc.sync.dma_start(ws[:], wsrc[ig].rearrange("dp (i q dt jr) -> dp i q dt jr", i=8, q=K, dt=2))
                for i8 in range(8):
                    i = 8 * ig + i8
                    p1 = p1p.tile([128, K, B], F32, tag="p1")
                    for q in range(K):
                        for dt in range(2):
                            nc.tensor.matmul(
                                p1[:, q, :],
                                ws[:, i8, q, dt, :],
                                hT16[dt][:, i, :],
                                start=(dt == 0), stop=(dt == 1),
                            )
                    nc.any.tensor_copy(inter[:, :, i, :], p1[:])
        else:
            # ---- einsum1: inter2[(il,r), ig, j, b] via column tile_position ----
            inter2 = sb.tile([128, 32, JL, B], BF16)
            for ig in range(32):
                ws = wsp.tile([128, 4, JL, 2, R], BF16, tag="ws")
                nc.sync.dma_start(ws[:], wsrc[ig].rearrange("dp (il j dt r) -> dp il j dt r", il=4, j=JL, dt=2))
                p1 = p1p.tile([128, JL, B], F32, tag="p1")
                for il in range(4):
                    i = 4 * ig + il
                    for j in range(JL):
                        for dt in range(2):
                            nc.tensor.matmul(
                                p1[32 * il : 32 * (il + 1), j, :],
                                ws[:, il, j, dt, :],
                                hT16[dt][:, i, :],
                                start=(dt == 0), stop=(dt == 1),
                                tile_position=(0, 32 * il),
                            )
                nc.vector.tensor_copy(inter2[:, ig, :, :], p1[:])

        # ---- einsum2 per quad q; relu + transpose into tcandT ----
        tcandT = sb.tile([128, 2, JL, B], F32)   # relu(infl)^T
        for q in range(K):
            # one PSUM bank per jl — concurrent row tiles must not share banks
            p2 = [p2p.tile([B, D], F32, tag=f"p2_{jl}", name=f"p2_{jl}")[:] for jl in range(4)]
            for gg in range(8):
                wt = wtp.tile([128, 4, 4, D], BF16, tag="wt")
                nc.sync.dma_start(wt[:], wtgt[q, gg].rearrange("p (gl il d) -> p gl il d", gl=4, il=4))
                if LAYOUT == "Q":
                    for gl in range(4):
                        for il in range(4):
                            i = 16 * gg + 4 * gl + il
                            for jl in range(4):
                                nc.tensor.matmul(
                                    p2[jl],
                                    inter[32 * jl : 32 * (jl + 1), q, i, :],
                                    wt[32 * jl : 32 * (jl + 1), gl, il, :],
                                    start=(i == 0), stop=(i == S - 1),
                                    tile_position=(32 * jl, 0),
                                )
                else:
                    for igl in range(4):
                        ig = 4 * gg + igl
                        for jl in range(4):
                            nc.tensor.matmul(
                                p2[jl],
                                inter2[:, ig, 4 * q + jl, :],
                                wt[:, igl, jl, :],
                                start=(ig == 0), stop=(ig == 31),
                            )
            hrelu = sb.tile([B, 4, D], F32, tag="hrelu")
            for jl in range(4):
                nc.scalar.activation(
                    hrelu[:, jl, :], p2[jl],
                    mybir.ActivationFunctionType.Relu,
                )
            for jloc in range(4):
                for dt in range(2):
                    p3 = p3p.tile([128, B], F32, tag="p3")
                    nc.tensor.transpose(
                        p3[:],
                        hrelu[:, jloc, 128 * dt : 128 * (dt + 1)],
                        ident[0:B, 0:B],
                    )
                    nc.vector.tensor_copy(
                        tcandT[:, dt, 4 * q + jloc, :], p3[:]
                    )

        if dbg is not None:
            nc.sync.dma_start(
                dbg[:, 256:512],
                tcandT[:].rearrange("p dt jl b -> p (dt jl b)"),
            )
        # ---- residual + LayerNorm in transposed layout (128 lanes) ----
        # hx holds x and x^2 side by side so ONE accumulation group computes
        # both partition sums (interleaved psum groups sharing a bank lose
        # the earlier group's partial on the later group's start).
        hx = sb.tile([128, 2, 2, JL, B], F32, tag="hx")  # [dp, dt, {x,x2}, jl, b]
        hcand = hx[:, :, 0, :, :]
        nc.vector.tensor_tensor(
            hcand, tcandT[:], hown32[:], op=mybir.AluOpType.add
        )
        if dbg is not None:
            nc.sync.dma_start(
                dbg[:, 512:768].rearrange("p (dt f) -> p dt f", dt=2),
                hcand.rearrange("p dt jl b -> p dt (jl b)"),
            )
        nc.vector.tensor_tensor(
            hx[:, :, 1, :, :], hcand, hcand, op=mybir.AluOpType.mult
        )
        s12 = p4p.tile([1, 2, JL * B], F32, tag="s12")
        for dt in range(2):
            nc.tensor.matmul(
                s12[:].rearrange("o c f -> o (c f)"), ones[:, 0:1],
                hx[:, dt, :, :, :].rearrange("p c jl b -> p (c jl b)"),
                start=(dt == 0), stop=(dt == 1),
            )
        mean = sb.tile([1, JL * B], F32, tag="mean")
        nc.vector.tensor_scalar_mul(mean[:], s12[:, 0, :], 1.0 / D)
        ex2 = sb.tile([1, JL * B], F32, tag="ex2")
        nc.vector.tensor_scalar_mul(ex2[:], s12[:, 1, :], 1.0 / D)
        m2 = sb.tile([1, JL * B], F32, tag="m2")
        nc.vector.tensor_tensor(m2[:], mean[:], mean[:], op=mybir.AluOpType.mult)
        var = sb.tile([1, JL * B], F32, tag="var")
        nc.vector.tensor_tensor(var[:], ex2[:], m2[:], op=mybir.AluOpType.subtract)
        std = sb.tile([1, JL * B], F32, tag="std")
        nc.scalar.activation(
            std[:], var[:], mybir.ActivationFunctionType.Sqrt,
            bias=eps_sb[0:1, :], scale=1.0,
        )
        # mean/rstd concat -> single broadcast matmul (one group)
        mr = sb.tile([1, 2, JL * B], F32, tag="mr")
        nc.vector.tensor_copy(mr[:, 0, :], mean[:])
        nc.vector.reciprocal(mr[:, 1, :], std[:])
        rstd = mr[:, 1, :]
        mbrb = p3p.tile([128, 2, JL * B], F32, tag="mbrb")
        nc.tensor.matmul(
            mbrb[:].rearrange("p c f -> p (c f)"), ones_row[:],
            mr[:].rearrange("o c f -> o (c f)"),
            start=True, stop=True,
        )
        t1 = sb.tile([128, 2, JL, B], F32, tag="t1")
        mb_r = mbrb[:, 0, :].rearrange("p (jl b) -> p jl b", jl=JL)
        rb_r = mbrb[:, 1, :].rearrange("p (jl b) -> p jl b", jl=JL)
        for dt in range(2):
            nc.vector.tensor_tensor(
                t1[:, dt, :, :], hcand[:, dt, :, :], mb_r,
                op=mybir.AluOpType.subtract,
            )
            nc.vector.tensor_tensor(
                t1[:, dt, :, :], t1[:, dt, :, :], rb_r,
                op=mybir.AluOpType.mult,
            )
            nc.vector.tensor_scalar(
                hown32[:, dt, :, :], t1[:, dt, :, :],
                lngT_sb[:, t, dt, 0:1], lngT_sb[:, t, dt, 1:2],
                op0=mybir.AluOpType.mult, op1=mybir.AluOpType.add,
            )
        nc.vector.tensor_copy(hown16[:], hown32[:])
        if dbg is not None:
            nc.sync.dma_start(
                dbg[:, 1024:1280],
                hown32[:].rearrange("p dt jl b -> p (dt jl b)"),
            )
            nc.sync.dma_start(dbg[0:1, 768:896], mean[:])
            nc.sync.dma_start(dbg[0:1, 896:1024], rstd)

        # ---- AllGather the own transposed slots (bf16); rebuild hT16 ----
        agin = drp.tile([2 * 128 * JL * B], BF16)
        for dt in range(2):
            nc.sync.dma_start(
                agin[dt * 16384 : (dt + 1) * 16384].rearrange(
                    "(p f) -> p f", p=128
                ),
                hown16[:, dt, :, :],
            )
        agout = drp.tile([NC, 2 * 128 * JL * B], BF16, addr_space="Shared")
        nc.gpsimd.collective_compute(
            "AllGather", mybir.AluOpType.bypass,
            ins=[agin[:].opt()], outs=[agout[:].opt()],
            replica_groups=[list(range(NC))],
        )
        agr = agout[:].rearrange(
            "rk (dt dp jl b) -> dt dp rk jl b", dt=2, dp=128, jl=JL, b=B
        )
        for dt in range(2):
            nc.sync.dma_start(
                hT16[dt][:].rearrange("dp (rk jl) b -> dp rk jl b", rk=NC), agr[dt]
            )


def _phase_c(nc, tc, io, ident, pid, hT16, qoT):
    with tc.tile_pool(name="pc_sb", bufs=1) as pc, \
         tc.tile_pool(name="pc_ps", bufs=3, space="PSUM") as cps, \
         tc.tile_pool(name="pc_lg", bufs=5, space="PSUM") as lgps, \
         tc.tile_pool(name="pc_wo", bufs=6) as wop:

        wkf_sb = [pc.tile([128, D], F32, name=f"wkf{ct}") for ct in range(2)]
        wvf_sb = [pc.tile([128, D], F32, name=f"wvf{ct}") for ct in range(2)]
        for ct in range(2):
            nc.sync.dma_start(
                wkf_sb[ct][:], io["wkfT"][128 * ct : 128 * (ct + 1), :]
            )
            nc.sync.dma_start(
                wvf_sb[ct][:], io["wvfT"][128 * ct : 128 * (ct + 1), :]
            )

        # own-batch h^T slice (dynamic b=pid) -> static f32 tiles
        pid_v = nc.vector.partition_id()
        hb = [pc.tile([128, S], F32, name=f"hb{dt}") for dt in range(2)]
        for dt in range(2):
            nc.vector.tensor_copy(
                hb[dt][:].rearrange("p (s o) -> p s o", o=1),
                hT16[dt][:, :, bass.ds(pid_v, 1)],
            )

        # K_f^T [d'128 x2, s128] ; V_f natural [s, d']
        kfT = [pc.tile([128, S], F32, name=f"kfT{pt}") for pt in range(2)]
        for pt in range(2):
            ps = cps.tile([128, L], F32, tag="c")
            for ct in range(2):
                nc.tensor.matmul(
                    ps[:, 0:S], wkf_sb[ct][:, 128 * pt : 128 * (pt + 1)], hb[ct][:],
                    start=(ct == 0), stop=(ct == 1),
                )
            nc.vector.tensor_copy(kfT[pt][:], ps[:, 0:S])
        vf = pc.tile([S, D], F32)
        psv = cps.tile([128, L], F32, tag="c")
        for ct in range(2):
            nc.tensor.matmul(
                psv[0:S, 0:D], hb[ct][:], wvf_sb[ct][:],
                start=(ct == 0), stop=(ct == 1),
            )
        nc.vector.tensor_copy(vf[:], psv[0:S, 0:D])

        # expand attention -> A2^T [s, t512]
        a2T = pc.tile([S, L], F32)
        for tt in range(4):
            sc = cps.tile([128, L], F32, tag="c")
            for pt in range(2):
                nc.tensor.matmul(
                    sc[:, 0:S], qoT[pt][:, 128 * tt : 128 * (tt + 1)], kfT[pt][:],
                    start=(pt == 0), stop=(pt == 1),
                )
            rowmax = pc.tile([128, 1], F32, tag="rmax2")
            nc.vector.tensor_reduce(
                rowmax[:], sc[:, 0:S], axis=mybir.AxisListType.X,
                op=mybir.AluOpType.max,
            )
            nb = pc.tile([128, 1], F32, tag="nb2")
            nc.vector.tensor_scalar_mul(nb[:], rowmax[:], -SCALE)
            a2 = pc.tile([128, S], F32, tag="a2")
            sumexp = pc.tile([128, 1], F32, tag="sexp2")
            nc.scalar.activation(
                a2[:], sc[:, 0:S], mybir.ActivationFunctionType.Exp,
                bias=nb[:], scale=SCALE, accum_out=sumexp[:],
            )
            rs = pc.tile([128, 1], F32, tag="rs2")
            nc.vector.reciprocal(rs[:], sumexp[:])
            nc.vector.tensor_scalar_mul(a2[:], a2[:], rs[:])
            ptr = cps.tile([128, L], F32, tag="c")
            nc.tensor.transpose(ptr[:, 0:S], a2[:], ident[:])
            nc.vector.tensor_copy(a2T[:, 128 * tt : 128 * (tt + 1)], ptr[:, 0:S])

        # Y^T [d128 x2, t512] -> bf16
        yT16 = [pc.tile([128, L], BF16, name=f"yT{dt}") for dt in range(2)]
        for dt in range(2):
            ps = cps.tile([128, L], F32, tag="c")
            nc.tensor.matmul(
                ps[:], vf[:, 128 * dt : 128 * (dt + 1)], a2T[:],
                start=True, stop=True,
            )
            nc.vector.tensor_copy(yT16[dt][:], ps[:])

        # logits tiles (bf16 matmul) + one coalesced bf16 store per vocab tile
        lg_dst = io["lg_out"].rearrange("(tt p) v -> p tt v", tt=4)
        for vt in range(NVT):
            wo_sb = wop.tile([128, 2, VT], BF16, tag="wo")
            nc.sync.dma_start(
                wo_sb[:], io["woutT"][vt].rearrange("dp (dt v) -> dp dt v", dt=2)
            )
            lg_sb = wop.tile([128, 4, VT], BF16, tag="lg_sb", name="lg_sb")
            for tt in range(4):
                lg = lgps.tile([128, VT], F32, tag="lg")
                for dt in range(2):
                    nc.tensor.matmul(
                        lg[:],
                        yT16[dt][:, 128 * tt : 128 * (tt + 1)],
                        wo_sb[:, dt, :],
                        start=(dt == 0), stop=(dt == 1),
                    )
                nc.any.tensor_copy(lg_sb[:, tt, :], lg[:])
            nc.sync.dma_start(
                lg_dst[:, :, VT * vt : VT * (vt + 1)], lg_sb[:]
            )


# ---------------------------------------------------------------------------
# Host side
# ---------------------------------------------------------------------------

_NC_CACHE = {}


def _get_nc():
    key = (N_STEPS, LAYOUT)
    if key not in _NC_CACHE:
        _NC_CACHE[key] = build()
    return _NC_CACHE[key]


def _prep_in_maps(inputs):
    f32 = lambda a: np.ascontiguousarray(np.asarray(a), dtype=np.float32)
    input_ids = np.asarray(inputs["input_ids"])
    attention_mask = np.asarray(inputs["attention_mask"])
    H = f32(inputs["H"])
    W_source = f32(inputs["W_source"])
    W_target = f32(inputs["W_target"])

    # lngT[dp, t, dt, {scale, bias}] = ln_{scale,bias}[t, 128dt+dp]
    lngT = np.stack(
        [
            f32(inputs["ln_scale"]).reshape(STEPS, 2, 128).transpose(2, 0, 1),
            f32(inputs["ln_bias"]).reshape(STEPS, 2, 128).transpose(2, 0, 1),
        ],
        axis=-1,
    )

    rep = {
        "wqT": f32(np.asarray(inputs["Wq_in"]).T),
        "wkslT": f32(np.asarray(inputs["Wk_slots"]).T),
        "wvT": f32(np.asarray(inputs["Wv_in"]).T),
        "wqoT": f32(np.asarray(inputs["Wq_out"]).T),
        "wkfT": f32(np.asarray(inputs["Wk_fin"]).T),
        "wvfT": f32(np.asarray(inputs["Wv_fin"]).T),
        "hT_in": f32(H.T),
        "hn_in": H,
        "lngT": np.ascontiguousarray(lngT),
        # woutT[vt, dp, (dt, vl)] = Wout[500vt+vl, 128dt+dp]  (bf16)
        "woutT": np.ascontiguousarray(
            f32(inputs["W_out_proj"]).reshape(NVT, VT, 2, 128)
            .transpose(0, 3, 2, 1).reshape(NVT, 128, 2 * VT)
        ).astype(NP_BF16),
    }

    in_maps = []
    for c in range(NC):
        m = dict(rep)
        X = (np.asarray(inputs["token_emb"], dtype=np.float32)[input_ids[c]]
             + np.asarray(inputs["pos_emb"], dtype=np.float32))
        m["xT_in"] = np.ascontiguousarray(X.T)
        m["maskw"] = np.ascontiguousarray(
            attention_mask[c].astype(np.float32).reshape(4, 128).T
        )
        ws = W_source[:, JL * c : JL * (c + 1)]      # [S, 16, D, R]
        # The reference masks out the i == j (diagonal) pair; zeroing
        # W_target[j, j] is exactly equivalent since the term is linear in it.
        wt = W_target[:, JL * c : JL * (c + 1)].copy()   # [S, 16, R, D]
        for jl in range(JL):
            wt[JL * c + jl, jl] = 0.0
        if LAYOUT == "Q":
            # ws[ig, dp, (i8, q, dt, jl, r)] = Ws[8ig+i8, 16c+4q+jl, 128dt+dp, r]
            ws = ws.reshape(16, 8, K, 4, 2, 128, R).transpose(0, 5, 1, 2, 4, 3, 6)
            m["wsrc"] = np.ascontiguousarray(ws).reshape(
                16, 128, 8 * K * 2 * 128
            ).astype(NP_BF16)
            # wt[q, gg, (jl, r), (gl, il, d)] = Wt[16gg+4gl+il, 16c+4q+jl, r, d]
            wt = wt.reshape(8, 4, 4, K, 4, R, D).transpose(3, 0, 4, 5, 1, 2, 6)
            m["wtgt"] = np.ascontiguousarray(wt).reshape(
                K, 8, 128, 4 * 4 * D
            ).astype(NP_BF16)
        else:
            # ws[ig, dp, (il, j, dt, r)] = Ws[4ig+il, 16c+j, 128dt+dp, r]
            ws = ws.reshape(32, 4, JL, 2, 128, R).transpose(0, 4, 1, 2, 3, 5)
            m["wsrc"] = np.ascontiguousarray(ws).reshape(
                32, 128, 4 * JL * 2 * R
            ).astype(NP_BF16)
            # wt[q, gg, (il, r), (igl, jl, d)] = Wt[16gg+4igl+il, 16c+4q+jl, r, d]
            wt = wt.reshape(8, 4, 4, K, 4, R, D).transpose(3, 0, 2, 5, 1, 4, 6)
            m["wtgt"] = np.ascontiguousarray(wt).reshape(
                K, 8, 128, 4 * 4 * D
            ).astype(NP_BF16)
        in_maps.append(m)
    return in_maps


def run(inputs, trace=False):
    nc = _get_nc()
    in_maps = _prep_in_maps(inputs)
    res = bass_utils.run_bass_kernel_spmd(
        nc, in_maps, core_ids=list(range(NC)), trace=trace
    )
    out = np.stack(
        [res.results[c]["lg_out"].astype(np.float32) for c in range(NC)], axis=0
    )
    return out, res


def kernel(**inputs):
    out, _ = run(inputs, trace=False)
    return out
